# revision 8
# baseline (speedup 1.0000x reference)
"""Trainium2 Bass kernel for nn_CNNModel_42064909697048.

Per-image row/col statistics (min/argmin/max/argmax/mean/median/argmedian
over both axes of each 28x28 image) -> 392 features -> 4-layer MLP ->
softmax, data-parallel over 8 NeuronCores.

Approach: values are packed into integer-exact fp32 keys
    key = 32*trunc(x*65536) + local_index
so a single min/max/rank-select on keys yields both the value and its
argindex (ties break toward the smaller index, matching numpy/torch).
Min, lower-median (rank 13) and max are produced simultaneously by one
Batcher odd-even sorting network pruned to outputs {0,13,27} (133
compare-exchanges), vectorized across 4 images x 28 groups per partition
in a position-major layout. Work is split across the Vector (axis-2 sort,
key build), GpSimd (axis-1 sort, sums) and Scalar (compare-exchange
copy-backs, activations) engines; the MLP runs on the tensor engine with
batch-512 matmuls. Index/scale corrections are folded into W1/b1.

Self-contained: hardcodes shapes/sharding; no sibling imports.
"""

import numpy as np

import concourse.bass as bass
import concourse.mybir as mybir
import concourse.tile as tile_mod
from concourse.tile import TileContext
from concourse.bass_utils import run_bass_kernel_spmd
from concourse.alu_op_type import AluOpType

# ---------------------------------------------------------------- constants
B_TOTAL = 131072
N_CORES = 8
B_CORE = B_TOTAL // N_CORES          # 16384
H = 28
D = 784
P = 128
PACK = 4                             # images per partition
TILE_IMGS = P * PACK                 # 512
N_TILES = B_CORE // TILE_IMGS        # 32
G = PACK * H                         # sort groups per partition = 112
FD = PACK * D                        # free dim of an image tile = 3136
NFEAT = 392
QS = 32768.0                         # value quantization scale (2^15)
KS = 32.0                            # index slots per quantum
C2X = float(2**23 + 2**19)           # RNE integerization bias (covers +-2^19)
F32 = mybir.dt.float32

# Batcher odd-even mergesort net for 28, pruned to outputs {0,13,27};
# stages of merged groups (d, start, (n1,s1), (n2,s2)):
# lo positions = {start + u*s1 + v*s2}, hi = lo + d.
NET28 = [[(1, 0, (14, 2), (1, 1))], [(2, 0, (7, 4), (2, 1))], [(1, 1, (7, 4), (1, 1)), (4, 0, (3, 8), (2, 3))], [(4, 1, (3, 8), (2, 1)), (8, 0, (2, 7), (1, 1)), (8, 16, (1, 1), (1, 1)), (1, 25, (1, 1), (1, 1))], [(2, 2, (3, 8), (2, 1)), (16, 0, (1, 1), (1, 1))], [(1, 1, (3, 8), (3, 2))], [(8, 1, (6, 1), (1, 1)), (8, 17, (3, 1), (1, 1)), (4, 20, (1, 1), (1, 1))], [(4, 4, (2, 17), (3, 1)), (4, 7, (1, 1), (1, 1)), (2, 18, (1, 1), (1, 1))], [(2, 2, (3, 4), (2, 1)), (2, 19, (2, 3), (1, 1)), (2, 23, (1, 1), (1, 1)), (1, 17, (1, 1), (1, 1))], [(1, 1, (2, 18), (4, 2)), (1, 9, (3, 2), (1, 1))], [(16, 1, (11, 1), (1, 1))], [(8, 8, (8, 1), (1, 1))], [(4, 7, (2, 5), (1, 1)), (4, 13, (2, 1), (1, 1)), (4, 23, (1, 1), (1, 1))], [(2, 11, (2, 3), (1, 1))], [(1, 13, (1, 1), (1, 1))]]

# feature column offsets within a 392-block (reference concat order)
OFF = {k: i * H for i, k in enumerate(
    ["min_v1", "min_i1", "min_v2", "min_i2",
     "max_v1", "max_i1", "max_v2", "max_i2",
     "mean_1", "mean_2",
     "med_v1", "med_i1", "med_v2", "med_i2"])}

# ------------------------------------------------- tile tail-drain workaround
def _patched_drain_and_barrier(self, tick_clock, wait_clock):
    drain_inst = self.nc.sync.drain()
    wait_clock.add_sem_waits(
        drain_inst.ins, tile_mod.ScopedClock({None: tick_clock.global_clock})
    )
    si = drain_inst.ins.sync_info
    waits = list(si.on_wait or [])
    if len(waits) > 1:
        si.on_wait = waits[:1]
        for w in waits[1:]:
            d2 = self.nc.sync.drain()
            si2 = d2.ins.sync_info
            if si2 is None:
                d2.ins.sync_info = mybir.SyncInfo(on_wait=[w], on_update=[])
            else:
                si2.on_wait = [w]
    self.nc.all_engine_barrier()
    assert self.sems is not None
    popped = self.nc._tile_sem_poison_stack.pop()
    assert popped is self._sem_poison
    self.nc.clear_and_free_semaphores(list(self.sems.allocated().values()))
    self.nc.all_engine_barrier()


tile_mod.TileContext._drain_and_barrier = _patched_drain_and_barrier


def _pos2d(base, n1, s1, n2, s2, d):
    """Return access plans for a merged CE group in a position-major
    [p, 28, G] view. Yields ('slc', lo_args, hi_args) per emitted op where
    args describe how to slice. Falls back to splitting when a 2D pattern
    isn't expressible as an einops view."""
    def ok1d(b, n, s):
        return (b, n, s)

    if n1 == 1 or n2 == 1:
        n, s = (n2, s2) if n1 == 1 else (n1, s1)
        yield ("1d", ok1d(base, n, s), ok1d(base + d, n, s))
        return
    # try 2D einops view: requires s2 == 1, s1 | 28, block fits
    def try2d(b):
        if s2 != 1 or 28 % s1 != 0:
            return None
        a0, b0 = b // s1, b % s1
        if b0 + n2 <= s1 and a0 + n1 <= 28 // s1:
            return (a0, b0)
        return None
    lo2, hi2 = try2d(base), try2d(base + d)
    if lo2 is not None and hi2 is not None:
        yield ("2d", (s1, lo2[0], lo2[1], n1, n2), (s1, hi2[0], hi2[1], n1, n2))
        return
    # split along the smaller axis into 1D ops
    if n1 <= n2:
        for u in range(n1):
            b = base + u * s1
            yield ("1d", ok1d(b, n2, s2), ok1d(b + d, n2, s2))
    else:
        for v in range(n2):
            b = base + v * s2
            yield ("1d", ok1d(b, n1, s1), ok1d(b + d, n1, s1))


# ------------------------------------------------------------- bass program
def build_nc(n_tiles: int = N_TILES, debug_features: bool = False):
    nc = bass.Bass()
    t_in = nc.dram_tensor("t", [TILE_IMGS * n_tiles, D], F32,
                          kind="ExternalInput")
    w1 = nc.dram_tensor("w1", [NFEAT, 270], F32, kind="ExternalInput")
    b1 = nc.dram_tensor("b1", [270, 1], F32, kind="ExternalInput")
    w2 = nc.dram_tensor("w2", [270, 90], F32, kind="ExternalInput")
    b2 = nc.dram_tensor("b2", [90, 1], F32, kind="ExternalInput")
    w3 = nc.dram_tensor("w3", [90, 30], F32, kind="ExternalInput")
    b3 = nc.dram_tensor("b3", [30, 1], F32, kind="ExternalInput")
    w4 = nc.dram_tensor("w4", [30, 10], F32, kind="ExternalInput")
    b4 = nc.dram_tensor("b4", [10, 1], F32, kind="ExternalInput")
    idn = nc.dram_tensor("idn", [P, P], F32, kind="ExternalInput")
    cg2 = nc.dram_tensor("cg2", [P, D], F32, kind="ExternalInput")  # col idx
    cg1 = nc.dram_tensor("cg1", [P, D], F32, kind="ExternalInput")  # row idx
    cbias = nc.dram_tensor("cbias", [P, 3], F32, kind="ExternalInput")
    wx = nc.dram_tensor("wx", [D, 270], F32, kind="ExternalInput")
    if debug_features:
        y_out = nc.dram_tensor("y", [TILE_IMGS * n_tiles, NFEAT], F32,
                               kind="ExternalOutput")
    else:
        y_out = nc.dram_tensor("y", [TILE_IMGS * n_tiles, 10], F32,
                               kind="ExternalOutput")

    MIN = AluOpType.min
    MAX = AluOpType.max
    ADD = AluOpType.add
    SUB = AluOpType.subtract
    MUL = AluOpType.mult
    MOD = AluOpType.mod
    AXX = mybir.AxisListType.X

    with TileContext(nc) as tc:
        with (
            tc.tile_pool(name="wpool", bufs=1) as wpool,
            tc.tile_pool(name="xpool", bufs=2) as xpool,
            tc.tile_pool(name="kpool", bufs=1) as kpool,
            tc.tile_pool(name="vpool", bufs=1) as vpool,
            tc.tile_pool(name="tspool", bufs=3) as tspool,
            tc.tile_pool(name="fpool", bufs=2) as fpool,
            tc.tile_pool(name="mpool", bufs=2) as mpool,
            tc.tile_pool(name="mxpool", bufs=1) as mxpool,
            tc.tile_pool(name="psT", bufs=2, space="PSUM") as psT,
            tc.tile_pool(name="psL", bufs=2, space="PSUM") as psL,
            tc.tile_pool(name="psS", bufs=2, space="PSUM") as psS,
        ):
            # ---- static weights/consts into SBUF
            w1_t = [wpool.tile([128, 270], F32, name=f"w1_{i}", tag=f"w1_{i}")
                    for i in range(3)]
            w1_t.append(wpool.tile([8, 270], F32, name="w1_3", tag="w1_3"))
            for i in range(3):
                nc.sync.dma_start(w1_t[i][:], w1[128 * i:128 * (i + 1), :])
            nc.sync.dma_start(w1_t[3][:], w1[384:392, :])
            w2_t = [wpool.tile([128, 90], F32, name="w2_0", tag="w2_0"),
                    wpool.tile([128, 90], F32, name="w2_1", tag="w2_1"),
                    wpool.tile([14, 90], F32, name="w2_2", tag="w2_2")]
            nc.sync.dma_start(w2_t[0][:], w2[0:128, :])
            nc.sync.dma_start(w2_t[1][:], w2[128:256, :])
            nc.sync.dma_start(w2_t[2][:], w2[256:270, :])
            w3_t = wpool.tile([90, 30], F32, name="w3", tag="w3")
            nc.sync.dma_start(w3_t[:], w3[:, :])
            w4_t = wpool.tile([30, 10], F32, name="w4", tag="w4")
            nc.sync.dma_start(w4_t[:], w4[:, :])
            b1_t = [wpool.tile([128, 1], F32, name="b1_0", tag="b1_0"),
                    wpool.tile([128, 1], F32, name="b1_1", tag="b1_1"),
                    wpool.tile([14, 1], F32, name="b1_2", tag="b1_2")]
            nc.sync.dma_start(b1_t[0][:], b1[0:128, :])
            nc.sync.dma_start(b1_t[1][:], b1[128:256, :])
            nc.sync.dma_start(b1_t[2][:], b1[256:270, :])
            b2_t = wpool.tile([90, 1], F32, name="b2", tag="b2")
            nc.sync.dma_start(b2_t[:], b2[:, :])
            b3_t = wpool.tile([30, 1], F32, name="b3", tag="b3")
            nc.sync.dma_start(b3_t[:], b3[:, :])
            b4_t = wpool.tile([10, 1], F32, name="b4", tag="b4")
            nc.sync.dma_start(b4_t[:], b4[:, :])
            idn_t = wpool.tile([P, P], F32, name="idn", tag="idn")
            nc.sync.dma_start(idn_t[:], idn[:, :])
            cg2_t = wpool.tile([P, D], F32, name="cg2", tag="cg2")
            nc.sync.dma_start(cg2_t[:], cg2[:, :])
            cg1_t = wpool.tile([P, D], F32, name="cg1", tag="cg1")
            nc.sync.dma_start(cg1_t[:], cg1[:, :])
            x_chunks = [(128 * j, 128) for j in range(6)] + [(768, 16)]
            wx_t = []
            for j, (x0, xc) in enumerate(x_chunks):
                wt = wpool.tile([xc, 270], F32, name=f"wx_{j}", tag=f"wx_{j}")
                nc.sync.dma_start(wt[:], wx[x0:x0 + xc, :])
                wx_t.append(wt)
            cb_t = wpool.tile([P, 3], F32, name="cbias", tag="cbias")
            nc.sync.dma_start(cb_t[:], cbias[:, :])
            bC = cb_t[:, 0:1]        # +C2X
            bNC = cb_t[:, 1:2]       # -C2X
            cg2_b = cg2_t[:].rearrange("p (u d) -> p u d", u=1).broadcast_to(
                [P, PACK, D])
            cg1_b = cg1_t[:].rearrange("p (u d) -> p u d", u=1).broadcast_to(
                [P, PACK, D])
            IDENT = mybir.ActivationFunctionType.Identity

            m1_chunks = [(0, 128), (128, 128), (256, 14)]
            k_chunks = [(0, 128), (128, 128), (256, 128), (384, 8)]

            for it in range(n_tiles):
                X = xpool.tile([P, FD], F32, name="x", tag="x")
                nc.sync.dma_start(
                    X[:], t_in[TILE_IMGS * it:TILE_IMGS * (it + 1), :]
                    .rearrange("(p i) d -> p (i d)", i=PACK))

                # ---- key build: A = 32*round(QS*x) + local_idx
                # round() via the RNE +-(2^23+2^19) trick on the fp32 adder.
                # Tt/Tu share the V buffers (dead before the sorts write them)
                Tt = vpool.tile([P, FD], F32, name="tt", tag="v2")
                Tu = vpool.tile([P, FD], F32, name="tu", tag="v1")
                A1 = kpool.tile([P, FD], F32, name="a1", tag="a1")
                A2 = kpool.tile([P, FD], F32, name="a2", tag="a2")
                nc.scalar.activation(Tt[:], X[:], IDENT, bias=bC, scale=QS)
                nc.scalar.activation(Tu[:], Tt[:], IDENT, bias=bNC, scale=1.0)
                Tui = Tu[:].rearrange("p (i d) -> p i d", d=D)
                A1i4 = A1[:].rearrange("p (i d) -> p i d", d=D)
                A2i4 = A2[:].rearrange("p (i d) -> p i d", d=D)
                nc.vector.scalar_tensor_tensor(A1i4, Tui, KS, cg1_b,
                                               op0=MUL, op1=ADD)
                nc.vector.scalar_tensor_tensor(A2i4, Tui, KS, cg2_b,
                                               op0=MUL, op1=ADD)

                V2 = vpool.tile([P, FD], F32, name="v2", tag="v2")
                V1 = vpool.tile([P, FD], F32, name="v1", tag="v1")
                V2x = V2[:].rearrange("p (x g) -> p x g", g=G)
                V1x = V1[:].rearrange("p (x g) -> p x g", g=G)

                # ---- stage 0 (writes every position; no temp/copy needed)
                # axis-2: groups (i,r) stride 28, positions c
                A2x = A2[:].rearrange("p (g x) -> p x g", x=H)
                nc.vector.tensor_tensor(V2x[:, 0:27:2, :], A2x[:, 0:27:2, :],
                                        A2x[:, 1:28:2, :], op=MIN)
                nc.vector.tensor_tensor(V2x[:, 1:28:2, :], A2x[:, 0:27:2, :],
                                        A2x[:, 1:28:2, :], op=MAX)
                # axis-1: groups (i,c), positions r (merged across images)
                A1p = A1[:].rearrange("p (i r c) -> p r i c", r=H, c=H)
                V1p = V1[:].rearrange("p (x i c) -> p x i c", i=PACK, c=H)
                nc.vector.tensor_tensor(V1p[:, 0:27:2, :, :],
                                        A1p[:, 0:27:2, :, :],
                                        A1p[:, 1:28:2, :, :], op=MIN)
                nc.vector.tensor_tensor(V1p[:, 1:28:2, :, :],
                                        A1p[:, 0:27:2, :, :],
                                        A1p[:, 1:28:2, :, :], op=MAX)

                # ---- mean features ride the l1 GEMM via Wx @ X^T; the F
                # mean columns are dead (W1_eff zeroed there) but must be
                # finite -> memset once on the idle gpsimd engine
                F = fpool.tile([P, PACK * NFEAT], F32, name="feat", tag="feat")
                Ff = F[:].rearrange("p (i f) -> p i f", f=NFEAT)
                o = OFF["mean_1"]
                nc.gpsimd.memzero(Ff[:, :, o:o + 2 * H])

                # ---- remaining sort stages, interleaved axis-2/axis-1
                def mk_view(Vt, spec, arg):
                    if spec == "1d":
                        b, n, s = arg
                        vx = Vt[:].rearrange("p (x g) -> p x g", g=G)
                        return vx[:, b:b + s * (n - 1) + 1:s, :] if s > 1 \
                            else vx[:, b:b + n, :]
                    s1, a0, b0, n1, n2 = arg
                    vab = Vt[:].rearrange("p (a b g) -> p a b g", b=s1, g=G)
                    return vab[:, a0:a0 + n1, b0:b0 + n2, :]

                def emit_stage(Vt, stage, eng, ts_tag):
                    for (d, st, (n1, s1), (n2, s2)) in stage:
                        for spec, lo_a, hi_a in _pos2d(st, n1, s1, n2, s2, d):
                            lo = mk_view(Vt, spec, lo_a)
                            hi = mk_view(Vt, spec, hi_a)
                            Ts = tspool.tile([P, 14 * G], F32, name="ts",
                                             tag=ts_tag)
                            if spec == "1d":
                                n = lo_a[1]
                                tt = Ts[:].rearrange(
                                    "p (s g) -> p s g", g=G)[:, 0:n, :]
                            else:
                                n1_, n2_ = lo_a[3], lo_a[4]
                                bb = 2 if n2_ <= 2 else 4
                                tt = Ts[:].rearrange(
                                    "p (a b g) -> p a b g", b=bb, g=G)[
                                        :, 0:n1_, 0:n2_, :]
                            eng.tensor_tensor(tt, lo, hi, op=MIN)
                            eng.tensor_tensor(hi, lo, hi, op=MAX)
                            nc.scalar.copy(lo, tt)

                for stage in NET28[1:]:
                    emit_stage(V2, stage, nc.vector, "ts2")
                    emit_stage(V1, stage, nc.vector, "ts1")

                # ---- feature extraction from key arrays
                # q = rne_int(K/32 - 15.5/32); Fv = q; Fi = K - 32q
                def extract(Vt, pos, vname, iname):
                    K = Vt[:, pos * G:(pos + 1) * G].rearrange(
                        "p (i r) -> p i r", r=H)
                    ov, oi = OFF[vname], OFF[iname]
                    Fv = Ff[:, :, ov:ov + H]
                    Fi = Ff[:, :, oi:oi + H]
                    t1 = mpool.tile([P, G], F32, name="ext1", tag="ext1")
                    t1v = t1[:].rearrange("p (i r) -> p i r", r=H)
                    nc.scalar.activation(t1v, K, IDENT, bias=bC,
                                         scale=1.0 / 32.0)
                    nc.scalar.activation(Fv, t1v, IDENT, bias=bNC, scale=1.0)
                    nc.vector.scalar_tensor_tensor(Fi, Fv, -KS, K,
                                                   op0=MUL, op1=ADD)

                extract(V2, 0, "min_v2", "min_i2")
                extract(V2, 13, "med_v2", "med_i2")
                extract(V2, 27, "max_v2", "max_i2")
                extract(V1, 0, "min_v1", "min_i1")
                extract(V1, 13, "med_v1", "med_i1")
                extract(V1, 27, "max_v1", "max_i1")

                if debug_features:
                    nc.sync.dma_start(
                        y_out[TILE_IMGS * it:TILE_IMGS * (it + 1), :]
                        .rearrange("(p i) f -> p (i f)", i=PACK), F[:])
                    continue

                # ---- MLP (batch 512 = 4 slots x 128 partitions)
                fTs = [mxpool.tile([128, TILE_IMGS], F32, name=f"fts{ci}",
                                  tag=f"fts{ci}") if kc == 128 else
                       mxpool.tile([8, TILE_IMGS], F32, name=f"fts{ci}",
                                  tag=f"fts{ci}")
                       for ci, (k0, kc) in enumerate(k_chunks)]
                xTs = [mxpool.tile([xc, TILE_IMGS], F32, name=f"xts{j}",
                                  tag=f"xts{j}")
                       for j, (x0, xc) in enumerate(x_chunks)]
                for i in range(PACK):
                    for ci, (k0, kc) in enumerate(k_chunks):
                        pt = psT.tile([P, P], F32, name=f"tp{i}_{ci}",
                                      tag="tp")
                        nc.tensor.transpose(
                            pt[0:kc, :], F[:, NFEAT * i + k0:NFEAT * i + k0 + kc],
                            idn_t[:])
                        nc.scalar.copy(fTs[ci][0:kc, 128 * i:128 * (i + 1)],
                                       pt[0:kc, :])
                    for j, (x0, xc) in enumerate(x_chunks):
                        pt = psT.tile([P, P], F32, name=f"xp{i}_{j}",
                                      tag="tp")
                        nc.tensor.transpose(
                            pt[0:xc, :], X[:, D * i + x0:D * i + x0 + xc],
                            idn_t[:])
                        nc.scalar.copy(xTs[j][0:xc, 128 * i:128 * (i + 1)],
                                       pt[0:xc, :])

                a1 = []
                for mi, (m0, mc) in enumerate(m1_chunks):
                    ps = psL.tile([P, TILE_IMGS], F32, name=f"l1_{m0}",
                                  tag="l1")[0:mc, :]
                    for ci, (k0, kc) in enumerate(k_chunks):
                        nc.tensor.matmul(ps[:], w1_t[ci][0:kc, m0:m0 + mc],
                                         fTs[ci][0:kc, :],
                                         start=(ci == 0), stop=False)
                    for j, (x0, xc) in enumerate(x_chunks):
                        nc.tensor.matmul(ps[:], wx_t[j][0:xc, m0:m0 + mc],
                                         xTs[j][0:xc, :],
                                         start=False, stop=(j == 6))
                    sb = mpool.tile([P, TILE_IMGS], F32, name=f"a1_{m0}",
                                    tag=f"a1_{m0}")[0:mc, :]
                    nc.scalar.activation(sb, ps,
                                         mybir.ActivationFunctionType.Relu,
                                         bias=b1_t[mi][0:mc, :], scale=1.0)
                    a1.append(sb)

                ps2 = psS.tile([P, TILE_IMGS], F32, name="l2",
                               tag="l2")[0:90, :]
                for ci, (k0, kc) in enumerate(m1_chunks):
                    nc.tensor.matmul(ps2[:], w2_t[ci][0:kc, :],
                                     a1[ci][0:kc, :] if kc != 128 else a1[ci],
                                     start=(ci == 0), stop=(ci == 2))
                a2t = mpool.tile([90, TILE_IMGS], F32, name="a2", tag="a2")
                nc.scalar.activation(a2t[:], ps2,
                                     mybir.ActivationFunctionType.Relu,
                                     bias=b2_t[:], scale=1.0)

                ps3 = psS.tile([P, TILE_IMGS], F32, name="l3",
                               tag="l3")[0:30, :]
                nc.tensor.matmul(ps3[:], w3_t[:], a2t[:], start=True, stop=True)
                a3t = mpool.tile([30, TILE_IMGS], F32, name="a3", tag="a3")
                nc.scalar.activation(a3t[:], ps3,
                                     mybir.ActivationFunctionType.Relu,
                                     bias=b3_t[:], scale=1.0)

                ps4 = psS.tile([P, TILE_IMGS], F32, name="l4",
                               tag="l2")[0:10, :]
                nc.tensor.matmul(ps4[:], w4_t[:], a3t[:], start=True, stop=True)
                ex = mpool.tile([10, TILE_IMGS], F32, name="ex", tag="ex")
                nc.scalar.activation(ex[:], ps4,
                                     mybir.ActivationFunctionType.Exp,
                                     bias=b4_t[:], scale=1.0)

                ext = mpool.tile([P, PACK * 10], F32, name="ext", tag="ext")
                for i in range(PACK):
                    pst = psT.tile([P, P], F32, name=f"sm{i}", tag="tp")
                    nc.tensor.transpose(pst[:, 0:10],
                                        ex[:, 128 * i:128 * (i + 1)],
                                        idn_t[0:10, 0:10])
                    nc.scalar.copy(ext[:, 10 * i:10 * (i + 1)], pst[:, 0:10])
                exi = ext[:].rearrange("p (i u) -> p i u", u=10)
                sums = mpool.tile([P, PACK], F32, name="sums", tag="sums")
                nc.vector.tensor_reduce(sums[:], exi, axis=AXX, op=ADD)
                rcp = mpool.tile([P, PACK], F32, name="rcp", tag="rcp")
                nc.vector.reciprocal(rcp[:], sums[:])
                yt = mpool.tile([P, PACK * 10], F32, name="yt", tag="yt")
                rcb = rcp[:].rearrange("p (i u) -> p i u", u=1).broadcast_to(
                    [P, PACK, 10])
                nc.vector.tensor_tensor(yt[:].rearrange("p (i u) -> p i u",
                                                        u=10),
                                        exi, rcb, op=MUL)
                nc.sync.dma_start(
                    y_out[TILE_IMGS * it:TILE_IMGS * (it + 1), :]
                    .rearrange("(p i) u -> p i u", i=PACK),
                    yt[:].rearrange("p (i u) -> p i u", u=10))

    _split_excess_waits(nc)
    return nc


MAX_WAITS = 1


def _split_excess_waits(nc):
    """Walrus in this container rejects instructions with >MAX_WAITS sem
    waits; hoist the excess onto NoOp carriers inserted just before."""
    import bass_rust
    ctr = [0]
    for f in nc.m.functions:
        for blk in f.blocks:
            insts = list(blk.instructions)
            out = []
            changed = False
            for inst in insts:
                si = inst.sync_info
                waits = list(si.on_wait) if (si and si.on_wait) else []
                if len(waits) > MAX_WAITS:
                    changed = True
                    excess = waits[:-MAX_WAITS]
                    si.on_wait = waits[-MAX_WAITS:]
                    for k in range(0, len(excess), MAX_WAITS):
                        nop = bass_rust.InstNoOp(
                            name=f"WSPLIT-{ctr[0]}", ins=[], outs=[])
                        ctr[0] += 1
                        nop.engine = inst.engine
                        nop.sync_info = mybir.SyncInfo(
                            on_wait=excess[k:k + MAX_WAITS], on_update=[])
                        out.append(nop)
                out.append(inst)
            if changed:
                blk.instructions = out


# ------------------------------------------------------------- numpy driver
def _prep_weights(W1, b1, W2, b2, W3, b3, W4, b4):
    """Fold per-feature affine corrections into W1/b1; return transposed
    weight matrices plus constant tiles."""
    scale = np.ones(NFEAT, np.float64)
    offset = np.zeros(NFEAT, np.float64)
    for name in ("min_v1", "min_v2", "max_v1", "max_v2",
                 "med_v1", "med_v2"):
        o = OFF[name]
        scale[o:o + H] = 1.0 / QS
    for name in ("mean_1", "mean_2"):
        o = OFF[name]
        scale[o:o + H] = 0.0           # means ride the Wx @ X^T GEMM
    for name in ("min_i1", "min_i2", "max_i1", "max_i2",
                 "med_i1", "med_i2"):
        o = OFF[name]
        offset[o:o + H] = 15.5
    W1_eff = W1.astype(np.float64) * scale[None, :]
    b1_eff = b1.astype(np.float64) + W1.astype(np.float64) @ offset
    gs = np.arange(D)
    rg, cg_ = gs // H, gs % H
    Wx = ((W1[:, OFF["mean_1"] + cg_].astype(np.float64)
           + W1[:, OFF["mean_2"] + rg].astype(np.float64)) / H).T
    c2 = np.tile(np.arange(H, dtype=np.float32) - 15.5, H)          # col idx
    c1 = np.repeat(np.arange(H, dtype=np.float32) - 15.5, H)         # row idx
    return {
        "w1": np.ascontiguousarray(W1_eff.T.astype(np.float32)),
        "b1": b1_eff.astype(np.float32).reshape(-1, 1),
        "w2": np.ascontiguousarray(W2.T.astype(np.float32)),
        "b2": b2.reshape(-1, 1).astype(np.float32),
        "w3": np.ascontiguousarray(W3.T.astype(np.float32)),
        "b3": b3.reshape(-1, 1).astype(np.float32),
        "w4": np.ascontiguousarray(W4.T.astype(np.float32)),
        "b4": b4.reshape(-1, 1).astype(np.float32),
        "idn": np.eye(P, dtype=np.float32),
        "cg2": np.broadcast_to(c2[None, :], (P, D)).copy(),
        "cg1": np.broadcast_to(c1[None, :], (P, D)).copy(),
        "cbias": np.broadcast_to(np.array(
            [C2X, -C2X, 0.0], np.float32)[None, :], (P, 3)).copy(),
        "wx": np.ascontiguousarray(Wx.astype(np.float32)),
    }


_NC_CACHE = {}


def _get_nc(n_tiles, debug_features, **kw):
    key = (n_tiles, debug_features, tuple(sorted(kw.items())))
    if key not in _NC_CACHE:
        _NC_CACHE[key] = build_nc(n_tiles, debug_features, **kw)
    return _NC_CACHE[key]


def run(t, weights, n_tiles=N_TILES, debug_features=False, trace=False, **kw):
    nc = _get_nc(n_tiles, debug_features, **kw)
    rows = TILE_IMGS * n_tiles
    in_maps = []
    for c in range(N_CORES):
        m = {"t": np.ascontiguousarray(t[c * B_CORE:c * B_CORE + rows])}
        m.update(weights)
        in_maps.append(m)
    res = run_bass_kernel_spmd(nc, in_maps, core_ids=list(range(N_CORES)),
                               trace=trace)
    outs = [r["y"] for r in res.results]
    return outs, res


def kernel(t, W1, b1, W2, b2, W3, b3, W4, b4):
    weights = _prep_weights(W1, b1, W2, b2, W3, b3, W4, b4)
    outs, _ = run(t, weights)
    y = np.concatenate(outs, axis=0)
    return np.ascontiguousarray(y.astype(np.float32))


# revision 9
# speedup vs baseline: 1.1715x; 1.1715x over previous
"""Trainium2 Bass kernel for nn_CNNModel_42064909697048.

Per-image row/col statistics (min/argmin/max/argmax/mean/median/argmedian
over both axes of each 28x28 image) -> 392 features -> 4-layer MLP ->
softmax, data-parallel over 8 NeuronCores.

Approach: values are packed into integer-exact fp32 keys
    key = 32*trunc(x*65536) + local_index
so a single min/max/rank-select on keys yields both the value and its
argindex (ties break toward the smaller index, matching numpy/torch).
Min, lower-median (rank 13) and max are produced simultaneously by one
Batcher odd-even sorting network pruned to outputs {0,13,27} (133
compare-exchanges), vectorized across 4 images x 28 groups per partition
in a position-major layout. Work is split across the Vector (axis-2 sort,
key build), GpSimd (axis-1 sort, sums) and Scalar (compare-exchange
copy-backs, activations) engines; the MLP runs on the tensor engine with
batch-512 matmuls. Index/scale corrections are folded into W1/b1.

Self-contained: hardcodes shapes/sharding; no sibling imports.
"""

import numpy as np

import concourse.bass as bass
import concourse.mybir as mybir
import concourse.tile as tile_mod
from concourse.tile import TileContext
from concourse.bass_utils import run_bass_kernel_spmd
from concourse.alu_op_type import AluOpType

# ---------------------------------------------------------------- constants
B_TOTAL = 131072
N_CORES = 8
B_CORE = B_TOTAL // N_CORES          # 16384
H = 28
D = 784
P = 128
PACK = 4                             # images per partition
TILE_IMGS = P * PACK                 # 512
N_TILES = B_CORE // TILE_IMGS        # 32
G = PACK * H                         # sort groups per partition = 112
FD = PACK * D                        # free dim of an image tile = 3136
NFEAT = 392
QS = 32768.0                         # value quantization scale (2^15)
KS = 32.0                            # index slots per quantum
C2X = float(2**23 + 2**19)           # RNE integerization bias (covers +-2^19)
F32 = mybir.dt.float32

# Batcher odd-even mergesort net for 28, pruned to outputs {0,13,27};
# stages of merged groups (d, start, (n1,s1), (n2,s2)):
# lo positions = {start + u*s1 + v*s2}, hi = lo + d.
NET28 = [[(1, 0, (14, 2), (1, 1))], [(2, 0, (7, 4), (2, 1))], [(1, 1, (7, 4), (1, 1)), (4, 0, (3, 8), (2, 3))], [(4, 1, (3, 8), (2, 1)), (8, 0, (2, 7), (1, 1)), (8, 16, (1, 1), (1, 1)), (1, 25, (1, 1), (1, 1))], [(2, 2, (3, 8), (2, 1)), (16, 0, (1, 1), (1, 1))], [(1, 1, (3, 8), (3, 2))], [(8, 1, (6, 1), (1, 1)), (8, 17, (3, 1), (1, 1)), (4, 20, (1, 1), (1, 1))], [(4, 4, (2, 17), (3, 1)), (4, 7, (1, 1), (1, 1)), (2, 18, (1, 1), (1, 1))], [(2, 2, (3, 4), (2, 1)), (2, 19, (2, 3), (1, 1)), (2, 23, (1, 1), (1, 1)), (1, 17, (1, 1), (1, 1))], [(1, 1, (2, 18), (4, 2)), (1, 9, (3, 2), (1, 1))], [(16, 1, (11, 1), (1, 1))], [(8, 8, (8, 1), (1, 1))], [(4, 7, (2, 5), (1, 1)), (4, 13, (2, 1), (1, 1)), (4, 23, (1, 1), (1, 1))], [(2, 11, (2, 3), (1, 1))], [(1, 13, (1, 1), (1, 1))]]

# feature column offsets within a 392-block (reference concat order)
OFF = {k: i * H for i, k in enumerate(
    ["min_v1", "min_i1", "min_v2", "min_i2",
     "max_v1", "max_i1", "max_v2", "max_i2",
     "mean_1", "mean_2",
     "med_v1", "med_i1", "med_v2", "med_i2"])}

# ------------------------------------------------- tile tail-drain workaround
def _patched_drain_and_barrier(self, tick_clock, wait_clock):
    drain_inst = self.nc.sync.drain()
    wait_clock.add_sem_waits(
        drain_inst.ins, tile_mod.ScopedClock({None: tick_clock.global_clock})
    )
    si = drain_inst.ins.sync_info
    waits = list(si.on_wait or [])
    if len(waits) > 1:
        si.on_wait = waits[:1]
        for w in waits[1:]:
            d2 = self.nc.sync.drain()
            si2 = d2.ins.sync_info
            if si2 is None:
                d2.ins.sync_info = mybir.SyncInfo(on_wait=[w], on_update=[])
            else:
                si2.on_wait = [w]
    self.nc.all_engine_barrier()
    assert self.sems is not None
    popped = self.nc._tile_sem_poison_stack.pop()
    assert popped is self._sem_poison
    self.nc.clear_and_free_semaphores(list(self.sems.allocated().values()))
    self.nc.all_engine_barrier()


tile_mod.TileContext._drain_and_barrier = _patched_drain_and_barrier


def _pos2d(base, n1, s1, n2, s2, d):
    """Return access plans for a merged CE group in a position-major
    [p, 28, G] view. Yields ('slc', lo_args, hi_args) per emitted op where
    args describe how to slice. Falls back to splitting when a 2D pattern
    isn't expressible as an einops view."""
    def ok1d(b, n, s):
        return (b, n, s)

    if n1 == 1 or n2 == 1:
        n, s = (n2, s2) if n1 == 1 else (n1, s1)
        yield ("1d", ok1d(base, n, s), ok1d(base + d, n, s))
        return
    # try 2D einops view: requires s2 == 1, s1 | 28, block fits
    def try2d(b):
        if s2 != 1 or 28 % s1 != 0:
            return None
        a0, b0 = b // s1, b % s1
        if b0 + n2 <= s1 and a0 + n1 <= 28 // s1:
            return (a0, b0)
        return None
    lo2, hi2 = try2d(base), try2d(base + d)
    if lo2 is not None and hi2 is not None:
        yield ("2d", (s1, lo2[0], lo2[1], n1, n2), (s1, hi2[0], hi2[1], n1, n2))
        return
    # split along the smaller axis into 1D ops
    if n1 <= n2:
        for u in range(n1):
            b = base + u * s1
            yield ("1d", ok1d(b, n2, s2), ok1d(b + d, n2, s2))
    else:
        for v in range(n2):
            b = base + v * s2
            yield ("1d", ok1d(b, n1, s1), ok1d(b + d, n1, s1))


# ------------------------------------------------------------- bass program
def build_nc(n_tiles: int = N_TILES, debug_features: bool = False):
    nc = bass.Bass()
    t_in = nc.dram_tensor("t", [TILE_IMGS * n_tiles, D], F32,
                          kind="ExternalInput")
    w1 = nc.dram_tensor("w1", [NFEAT, 270], F32, kind="ExternalInput")
    b1 = nc.dram_tensor("b1", [270, 1], F32, kind="ExternalInput")
    w2 = nc.dram_tensor("w2", [270, 90], F32, kind="ExternalInput")
    b2 = nc.dram_tensor("b2", [90, 1], F32, kind="ExternalInput")
    w3 = nc.dram_tensor("w3", [90, 30], F32, kind="ExternalInput")
    b3 = nc.dram_tensor("b3", [30, 1], F32, kind="ExternalInput")
    w4 = nc.dram_tensor("w4", [30, 10], F32, kind="ExternalInput")
    b4 = nc.dram_tensor("b4", [10, 1], F32, kind="ExternalInput")
    idn = nc.dram_tensor("idn", [P, P], F32, kind="ExternalInput")
    cg2 = nc.dram_tensor("cg2", [P, D], F32, kind="ExternalInput")  # col idx
    cg1 = nc.dram_tensor("cg1", [P, D], F32, kind="ExternalInput")  # row idx
    cbias = nc.dram_tensor("cbias", [P, 3], F32, kind="ExternalInput")
    if debug_features:
        y_out = nc.dram_tensor("y", [TILE_IMGS * n_tiles, NFEAT], F32,
                               kind="ExternalOutput")
    else:
        y_out = nc.dram_tensor("y", [TILE_IMGS * n_tiles, 10], F32,
                               kind="ExternalOutput")

    MIN = AluOpType.min
    MAX = AluOpType.max
    ADD = AluOpType.add
    SUB = AluOpType.subtract
    MUL = AluOpType.mult
    MOD = AluOpType.mod
    AXX = mybir.AxisListType.X

    with TileContext(nc) as tc:
        with (
            tc.tile_pool(name="wpool", bufs=1) as wpool,
            tc.tile_pool(name="xpool", bufs=2) as xpool,
            tc.tile_pool(name="kpool", bufs=1) as kpool,
            tc.tile_pool(name="vpool", bufs=1) as vpool,
            tc.tile_pool(name="tspool", bufs=3) as tspool,
            tc.tile_pool(name="fpool", bufs=2) as fpool,
            tc.tile_pool(name="mpool", bufs=2) as mpool,
            tc.tile_pool(name="mxpool", bufs=1) as mxpool,
            tc.tile_pool(name="psT", bufs=2, space="PSUM") as psT,
            tc.tile_pool(name="psL", bufs=2, space="PSUM") as psL,
            tc.tile_pool(name="psS", bufs=2, space="PSUM") as psS,
        ):
            # ---- static weights/consts into SBUF
            w1_t = [wpool.tile([128, 270], F32, name=f"w1_{i}", tag=f"w1_{i}")
                    for i in range(3)]
            w1_t.append(wpool.tile([8, 270], F32, name="w1_3", tag="w1_3"))
            for i in range(3):
                nc.sync.dma_start(w1_t[i][:], w1[128 * i:128 * (i + 1), :])
            nc.sync.dma_start(w1_t[3][:], w1[384:392, :])
            w2_t = [wpool.tile([128, 90], F32, name="w2_0", tag="w2_0"),
                    wpool.tile([128, 90], F32, name="w2_1", tag="w2_1"),
                    wpool.tile([14, 90], F32, name="w2_2", tag="w2_2")]
            nc.sync.dma_start(w2_t[0][:], w2[0:128, :])
            nc.sync.dma_start(w2_t[1][:], w2[128:256, :])
            nc.sync.dma_start(w2_t[2][:], w2[256:270, :])
            w3_t = wpool.tile([90, 30], F32, name="w3", tag="w3")
            nc.sync.dma_start(w3_t[:], w3[:, :])
            w4_t = wpool.tile([30, 10], F32, name="w4", tag="w4")
            nc.sync.dma_start(w4_t[:], w4[:, :])
            b1_t = [wpool.tile([128, 1], F32, name="b1_0", tag="b1_0"),
                    wpool.tile([128, 1], F32, name="b1_1", tag="b1_1"),
                    wpool.tile([14, 1], F32, name="b1_2", tag="b1_2")]
            nc.sync.dma_start(b1_t[0][:], b1[0:128, :])
            nc.sync.dma_start(b1_t[1][:], b1[128:256, :])
            nc.sync.dma_start(b1_t[2][:], b1[256:270, :])
            b2_t = wpool.tile([90, 1], F32, name="b2", tag="b2")
            nc.sync.dma_start(b2_t[:], b2[:, :])
            b3_t = wpool.tile([30, 1], F32, name="b3", tag="b3")
            nc.sync.dma_start(b3_t[:], b3[:, :])
            b4_t = wpool.tile([10, 1], F32, name="b4", tag="b4")
            nc.sync.dma_start(b4_t[:], b4[:, :])
            idn_t = wpool.tile([P, P], F32, name="idn", tag="idn")
            nc.sync.dma_start(idn_t[:], idn[:, :])
            cg2_t = wpool.tile([P, D], F32, name="cg2", tag="cg2")
            nc.sync.dma_start(cg2_t[:], cg2[:, :])
            cg1_t = wpool.tile([P, D], F32, name="cg1", tag="cg1")
            nc.sync.dma_start(cg1_t[:], cg1[:, :])
            cb_t = wpool.tile([P, 3], F32, name="cbias", tag="cbias")
            nc.sync.dma_start(cb_t[:], cbias[:, :])
            bC = cb_t[:, 0:1]        # +C2X
            bNC = cb_t[:, 1:2]       # -C2X
            cg2_b = cg2_t[:].rearrange("p (u d) -> p u d", u=1).broadcast_to(
                [P, PACK, D])
            cg1_b = cg1_t[:].rearrange("p (u d) -> p u d", u=1).broadcast_to(
                [P, PACK, D])
            IDENT = mybir.ActivationFunctionType.Identity

            m1_chunks = [(0, 128), (128, 128), (256, 14)]
            k_chunks = [(0, 128), (128, 128), (256, 128), (384, 8)]

            for it in range(n_tiles):
                X = xpool.tile([P, FD], F32, name="x", tag="x")
                nc.sync.dma_start(
                    X[:], t_in[TILE_IMGS * it:TILE_IMGS * (it + 1), :]
                    .rearrange("(p i) d -> p (i d)", i=PACK))

                # ---- key build: A = 32*round(QS*x) + local_idx
                # round() via the RNE +-(2^23+2^19) trick on the fp32 adder.
                # Tt/Tu share the V buffers (dead before the sorts write them)
                Tt = vpool.tile([P, FD], F32, name="tt", tag="v2")
                Tu = vpool.tile([P, FD], F32, name="tu", tag="v1")
                A1 = kpool.tile([P, FD], F32, name="a1", tag="a1")
                A2 = kpool.tile([P, FD], F32, name="a2", tag="a2")
                nc.scalar.activation(Tt[:], X[:], IDENT, bias=bC, scale=QS)
                nc.scalar.activation(Tu[:], Tt[:], IDENT, bias=bNC, scale=1.0)
                Tui = Tu[:].rearrange("p (i d) -> p i d", d=D)
                A1i4 = A1[:].rearrange("p (i d) -> p i d", d=D)
                A2i4 = A2[:].rearrange("p (i d) -> p i d", d=D)
                nc.vector.scalar_tensor_tensor(A1i4, Tui, KS, cg1_b,
                                               op0=MUL, op1=ADD)
                nc.vector.scalar_tensor_tensor(A2i4, Tui, KS, cg2_b,
                                               op0=MUL, op1=ADD)

                V2 = vpool.tile([P, FD], F32, name="v2", tag="v2")
                V1 = vpool.tile([P, FD], F32, name="v1", tag="v1")
                V2x = V2[:].rearrange("p (x g) -> p x g", g=G)
                V1x = V1[:].rearrange("p (x g) -> p x g", g=G)

                # ---- stage 0 (writes every position; no temp/copy needed)
                # axis-2: groups (i,r) stride 28, positions c
                A2x = A2[:].rearrange("p (g x) -> p x g", x=H)
                nc.vector.tensor_tensor(V2x[:, 0:27:2, :], A2x[:, 0:27:2, :],
                                        A2x[:, 1:28:2, :], op=MIN)
                nc.vector.tensor_tensor(V2x[:, 1:28:2, :], A2x[:, 0:27:2, :],
                                        A2x[:, 1:28:2, :], op=MAX)
                # axis-1: groups (i,c), positions r (merged across images)
                A1p = A1[:].rearrange("p (i r c) -> p r i c", r=H, c=H)
                V1p = V1[:].rearrange("p (x i c) -> p x i c", i=PACK, c=H)
                nc.vector.tensor_tensor(V1p[:, 0:27:2, :, :],
                                        A1p[:, 0:27:2, :, :],
                                        A1p[:, 1:28:2, :, :], op=MIN)
                nc.vector.tensor_tensor(V1p[:, 1:28:2, :, :],
                                        A1p[:, 0:27:2, :, :],
                                        A1p[:, 1:28:2, :, :], op=MAX)

                # ---- sums (mean features) while sorts run
                F = fpool.tile([P, PACK * NFEAT], F32, name="feat", tag="feat")
                Ff = F[:].rearrange("p (i f) -> p i f", f=NFEAT)
                Xi = X[:].rearrange("p (i r c) -> p i r c", r=H, c=H)
                Xi_t = X[:].rearrange("p (i r c) -> p i c r", r=H, c=H)
                o = OFF["mean_1"]
                nc.vector.tensor_reduce(Ff[:, :, o:o + H], Xi_t, axis=AXX,
                                        op=ADD)
                o = OFF["mean_2"]
                nc.vector.tensor_reduce(Ff[:, :, o:o + H], Xi, axis=AXX,
                                        op=ADD)

                # ---- remaining sort stages, interleaved axis-2/axis-1
                def mk_view(Vt, spec, arg):
                    if spec == "1d":
                        b, n, s = arg
                        vx = Vt[:].rearrange("p (x g) -> p x g", g=G)
                        return vx[:, b:b + s * (n - 1) + 1:s, :] if s > 1 \
                            else vx[:, b:b + n, :]
                    s1, a0, b0, n1, n2 = arg
                    vab = Vt[:].rearrange("p (a b g) -> p a b g", b=s1, g=G)
                    return vab[:, a0:a0 + n1, b0:b0 + n2, :]

                def emit_stage(Vt, stage, eng, ts_tag):
                    for (d, st, (n1, s1), (n2, s2)) in stage:
                        for spec, lo_a, hi_a in _pos2d(st, n1, s1, n2, s2, d):
                            lo = mk_view(Vt, spec, lo_a)
                            hi = mk_view(Vt, spec, hi_a)
                            Ts = tspool.tile([P, 14 * G], F32, name="ts",
                                             tag=ts_tag)
                            if spec == "1d":
                                n = lo_a[1]
                                tt = Ts[:].rearrange(
                                    "p (s g) -> p s g", g=G)[:, 0:n, :]
                            else:
                                n1_, n2_ = lo_a[3], lo_a[4]
                                bb = 2 if n2_ <= 2 else 4
                                tt = Ts[:].rearrange(
                                    "p (a b g) -> p a b g", b=bb, g=G)[
                                        :, 0:n1_, 0:n2_, :]
                            eng.tensor_tensor(tt, lo, hi, op=MIN)
                            eng.tensor_tensor(hi, lo, hi, op=MAX)
                            nc.scalar.copy(lo, tt)

                for stage in NET28[1:]:
                    emit_stage(V2, stage, nc.vector, "ts2")
                    emit_stage(V1, stage, nc.vector, "ts1")

                # ---- feature extraction from key arrays
                # q = rne_int(K/32 - 15.5/32); Fv = q; Fi = K - 32q
                def extract(Vt, pos, vname, iname):
                    K = Vt[:, pos * G:(pos + 1) * G].rearrange(
                        "p (i r) -> p i r", r=H)
                    ov, oi = OFF[vname], OFF[iname]
                    Fv = Ff[:, :, ov:ov + H]
                    Fi = Ff[:, :, oi:oi + H]
                    t1 = mpool.tile([P, G], F32, name="ext1", tag="ext1")
                    t1v = t1[:].rearrange("p (i r) -> p i r", r=H)
                    nc.scalar.activation(t1v, K, IDENT, bias=bC,
                                         scale=1.0 / 32.0)
                    nc.scalar.activation(Fv, t1v, IDENT, bias=bNC, scale=1.0)
                    nc.vector.scalar_tensor_tensor(Fi, Fv, -KS, K,
                                                   op0=MUL, op1=ADD)

                extract(V2, 0, "min_v2", "min_i2")
                extract(V2, 13, "med_v2", "med_i2")
                extract(V2, 27, "max_v2", "max_i2")
                extract(V1, 0, "min_v1", "min_i1")
                extract(V1, 13, "med_v1", "med_i1")
                extract(V1, 27, "max_v1", "max_i1")

                if debug_features:
                    nc.sync.dma_start(
                        y_out[TILE_IMGS * it:TILE_IMGS * (it + 1), :]
                        .rearrange("(p i) f -> p (i f)", i=PACK), F[:])
                    continue

                # ---- MLP (batch 512 = 4 slots x 128 partitions)
                fTs = [mxpool.tile([128, TILE_IMGS], F32, name=f"fts{ci}",
                                  tag=f"fts{ci}") if kc == 128 else
                       mxpool.tile([8, TILE_IMGS], F32, name=f"fts{ci}",
                                  tag=f"fts{ci}")
                       for ci, (k0, kc) in enumerate(k_chunks)]

                for i in range(PACK):
                    for ci, (k0, kc) in enumerate(k_chunks):
                        pt = psT.tile([P, P], F32, name=f"tp{i}_{ci}",
                                      tag="tp")
                        nc.tensor.transpose(
                            pt[0:kc, :], F[:, NFEAT * i + k0:NFEAT * i + k0 + kc],
                            idn_t[:])
                        nc.scalar.copy(fTs[ci][0:kc, 128 * i:128 * (i + 1)],
                                       pt[0:kc, :])


                a1 = []
                for mi, (m0, mc) in enumerate(m1_chunks):
                    ps = psL.tile([P, TILE_IMGS], F32, name=f"l1_{m0}",
                                  tag="l1")[0:mc, :]
                    for ci, (k0, kc) in enumerate(k_chunks):
                        nc.tensor.matmul(ps[:], w1_t[ci][0:kc, m0:m0 + mc],
                                         fTs[ci][0:kc, :],
                                         start=(ci == 0), stop=(ci == 3))
                    sb = mpool.tile([P, TILE_IMGS], F32, name=f"a1_{m0}",
                                    tag=f"a1_{m0}")[0:mc, :]
                    nc.scalar.activation(sb, ps,
                                         mybir.ActivationFunctionType.Relu,
                                         bias=b1_t[mi][0:mc, :], scale=1.0)
                    a1.append(sb)

                ps2 = psS.tile([P, TILE_IMGS], F32, name="l2",
                               tag="l2")[0:90, :]
                for ci, (k0, kc) in enumerate(m1_chunks):
                    nc.tensor.matmul(ps2[:], w2_t[ci][0:kc, :],
                                     a1[ci][0:kc, :] if kc != 128 else a1[ci],
                                     start=(ci == 0), stop=(ci == 2))
                a2t = mpool.tile([90, TILE_IMGS], F32, name="a2", tag="a2")
                nc.scalar.activation(a2t[:], ps2,
                                     mybir.ActivationFunctionType.Relu,
                                     bias=b2_t[:], scale=1.0)

                ps3 = psS.tile([P, TILE_IMGS], F32, name="l3",
                               tag="l3")[0:30, :]
                nc.tensor.matmul(ps3[:], w3_t[:], a2t[:], start=True, stop=True)
                a3t = mpool.tile([30, TILE_IMGS], F32, name="a3", tag="a3")
                nc.scalar.activation(a3t[:], ps3,
                                     mybir.ActivationFunctionType.Relu,
                                     bias=b3_t[:], scale=1.0)

                ps4 = psS.tile([P, TILE_IMGS], F32, name="l4",
                               tag="l2")[0:10, :]
                nc.tensor.matmul(ps4[:], w4_t[:], a3t[:], start=True, stop=True)
                ex = mpool.tile([10, TILE_IMGS], F32, name="ex", tag="ex")
                nc.scalar.activation(ex[:], ps4,
                                     mybir.ActivationFunctionType.Exp,
                                     bias=b4_t[:], scale=1.0)

                ext = mpool.tile([P, PACK * 10], F32, name="ext", tag="ext")
                for i in range(PACK):
                    pst = psT.tile([P, P], F32, name=f"sm{i}", tag="tp")
                    nc.tensor.transpose(pst[:, 0:10],
                                        ex[:, 128 * i:128 * (i + 1)],
                                        idn_t[0:10, 0:10])
                    nc.scalar.copy(ext[:, 10 * i:10 * (i + 1)], pst[:, 0:10])
                exi = ext[:].rearrange("p (i u) -> p i u", u=10)
                sums = mpool.tile([P, PACK], F32, name="sums", tag="sums")
                nc.vector.tensor_reduce(sums[:], exi, axis=AXX, op=ADD)
                rcp = mpool.tile([P, PACK], F32, name="rcp", tag="rcp")
                nc.vector.reciprocal(rcp[:], sums[:])
                yt = mpool.tile([P, PACK * 10], F32, name="yt", tag="yt")
                rcb = rcp[:].rearrange("p (i u) -> p i u", u=1).broadcast_to(
                    [P, PACK, 10])
                nc.vector.tensor_tensor(yt[:].rearrange("p (i u) -> p i u",
                                                        u=10),
                                        exi, rcb, op=MUL)
                nc.sync.dma_start(
                    y_out[TILE_IMGS * it:TILE_IMGS * (it + 1), :]
                    .rearrange("(p i) u -> p i u", i=PACK),
                    yt[:].rearrange("p (i u) -> p i u", u=10))

    _split_excess_waits(nc)
    return nc


MAX_WAITS = 1


def _split_excess_waits(nc):
    """Walrus in this container rejects instructions with >MAX_WAITS sem
    waits; hoist the excess onto NoOp carriers inserted just before."""
    import bass_rust
    ctr = [0]
    for f in nc.m.functions:
        for blk in f.blocks:
            insts = list(blk.instructions)
            out = []
            changed = False
            for inst in insts:
                si = inst.sync_info
                waits = list(si.on_wait) if (si and si.on_wait) else []
                if len(waits) > MAX_WAITS:
                    changed = True
                    excess = waits[:-MAX_WAITS]
                    si.on_wait = waits[-MAX_WAITS:]
                    for k in range(0, len(excess), MAX_WAITS):
                        nop = bass_rust.InstNoOp(
                            name=f"WSPLIT-{ctr[0]}", ins=[], outs=[])
                        ctr[0] += 1
                        nop.engine = inst.engine
                        nop.sync_info = mybir.SyncInfo(
                            on_wait=excess[k:k + MAX_WAITS], on_update=[])
                        out.append(nop)
                out.append(inst)
            if changed:
                blk.instructions = out


# ------------------------------------------------------------- numpy driver
def _prep_weights(W1, b1, W2, b2, W3, b3, W4, b4):
    """Fold per-feature affine corrections into W1/b1; return transposed
    weight matrices plus constant tiles."""
    scale = np.ones(NFEAT, np.float64)
    offset = np.zeros(NFEAT, np.float64)
    for name in ("min_v1", "min_v2", "max_v1", "max_v2",
                 "med_v1", "med_v2"):
        o = OFF[name]
        scale[o:o + H] = 1.0 / QS
    for name in ("mean_1", "mean_2"):
        o = OFF[name]
        scale[o:o + H] = 1.0 / H
    for name in ("min_i1", "min_i2", "max_i1", "max_i2",
                 "med_i1", "med_i2"):
        o = OFF[name]
        offset[o:o + H] = 15.5
    W1_eff = W1.astype(np.float64) * scale[None, :]
    b1_eff = b1.astype(np.float64) + W1.astype(np.float64) @ offset
    c2 = np.tile(np.arange(H, dtype=np.float32) - 15.5, H)          # col idx
    c1 = np.repeat(np.arange(H, dtype=np.float32) - 15.5, H)         # row idx
    return {
        "w1": np.ascontiguousarray(W1_eff.T.astype(np.float32)),
        "b1": b1_eff.astype(np.float32).reshape(-1, 1),
        "w2": np.ascontiguousarray(W2.T.astype(np.float32)),
        "b2": b2.reshape(-1, 1).astype(np.float32),
        "w3": np.ascontiguousarray(W3.T.astype(np.float32)),
        "b3": b3.reshape(-1, 1).astype(np.float32),
        "w4": np.ascontiguousarray(W4.T.astype(np.float32)),
        "b4": b4.reshape(-1, 1).astype(np.float32),
        "idn": np.eye(P, dtype=np.float32),
        "cg2": np.broadcast_to(c2[None, :], (P, D)).copy(),
        "cg1": np.broadcast_to(c1[None, :], (P, D)).copy(),
        "cbias": np.broadcast_to(np.array(
            [C2X, -C2X, 0.0], np.float32)[None, :], (P, 3)).copy(),
    }


_NC_CACHE = {}


def _get_nc(n_tiles, debug_features, **kw):
    key = (n_tiles, debug_features, tuple(sorted(kw.items())))
    if key not in _NC_CACHE:
        _NC_CACHE[key] = build_nc(n_tiles, debug_features, **kw)
    return _NC_CACHE[key]


def run(t, weights, n_tiles=N_TILES, debug_features=False, trace=False, **kw):
    nc = _get_nc(n_tiles, debug_features, **kw)
    rows = TILE_IMGS * n_tiles
    in_maps = []
    for c in range(N_CORES):
        m = {"t": np.ascontiguousarray(t[c * B_CORE:c * B_CORE + rows])}
        m.update(weights)
        in_maps.append(m)
    res = run_bass_kernel_spmd(nc, in_maps, core_ids=list(range(N_CORES)),
                               trace=trace)
    outs = [r["y"] for r in res.results]
    return outs, res


def kernel(t, W1, b1, W2, b2, W3, b3, W4, b4):
    weights = _prep_weights(W1, b1, W2, b2, W3, b3, W4, b4)
    outs, _ = run(t, weights)
    y = np.concatenate(outs, axis=0)
    return np.ascontiguousarray(y.astype(np.float32))


# revision 22
# speedup vs baseline: 1.1997x; 1.0241x over previous
"""Trainium2 Bass kernel for nn_CNNModel_42064909697048.

Per-image row/col statistics (min/argmin/max/argmax/mean/median/argmedian
over both axes of each 28x28 image) -> 392 features -> 4-layer MLP ->
softmax, data-parallel over 8 NeuronCores.

Approach: values are packed into integer-exact fp32 keys
    key = 32*trunc(x*65536) + local_index
so a single min/max/rank-select on keys yields both the value and its
argindex (ties break toward the smaller index, matching numpy/torch).
Min, lower-median (rank 13) and max are produced simultaneously by one
Batcher odd-even sorting network pruned to outputs {0,13,27} (133
compare-exchanges), vectorized across 4 images x 28 groups per partition
in a position-major layout. Work is split across the Vector (axis-2 sort,
key build), GpSimd (axis-1 sort, sums) and Scalar (compare-exchange
copy-backs, activations) engines; the MLP runs on the tensor engine with
batch-512 matmuls. Index/scale corrections are folded into W1/b1.

Self-contained: hardcodes shapes/sharding; no sibling imports.
"""

import numpy as np

import concourse.bass as bass
import concourse.mybir as mybir
import concourse.tile as tile_mod
from concourse.tile import TileContext
from concourse.bass_utils import run_bass_kernel_spmd
from concourse.alu_op_type import AluOpType

# ---------------------------------------------------------------- constants
B_TOTAL = 131072
N_CORES = 8
B_CORE = B_TOTAL // N_CORES          # 16384
H = 28
D = 784
P = 128
PACK = 8                             # images per partition
TILE_IMGS = P * PACK                 # 512
N_TILES = B_CORE // TILE_IMGS        # 32
G = PACK * H                         # sort groups per partition = 112
FD = PACK * D                        # free dim of an image tile = 3136
NFEAT = 392
QS = 32768.0                         # value quantization scale (2^15)
KS = 32.0                            # index slots per quantum
C2X = float(2**23 + 2**19)           # RNE integerization bias (covers +-2^19)
F32 = mybir.dt.float32

# Batcher odd-even mergesort net for 28, pruned to outputs {0,13,27};
# stages of merged groups (d, start, (n1,s1), (n2,s2)):
# lo positions = {start + u*s1 + v*s2}, hi = lo + d.
NET28 = [[(1, 0, (14, 2), (1, 1))], [(2, 0, (7, 4), (2, 1))], [(1, 1, (7, 4), (1, 1)), (4, 0, (3, 8), (2, 3))], [(4, 1, (3, 8), (2, 1)), (8, 0, (2, 7), (1, 1)), (8, 16, (1, 1), (1, 1)), (1, 25, (1, 1), (1, 1))], [(2, 2, (3, 8), (2, 1)), (16, 0, (1, 1), (1, 1))], [(1, 1, (3, 8), (3, 2))], [(8, 1, (6, 1), (1, 1)), (8, 17, (3, 1), (1, 1)), (4, 20, (1, 1), (1, 1))], [(4, 4, (2, 17), (3, 1)), (4, 7, (1, 1), (1, 1)), (2, 18, (1, 1), (1, 1))], [(2, 2, (3, 4), (2, 1)), (2, 19, (2, 3), (1, 1)), (2, 23, (1, 1), (1, 1)), (1, 17, (1, 1), (1, 1))], [(1, 1, (2, 18), (4, 2)), (1, 9, (3, 2), (1, 1))], [(16, 1, (11, 1), (1, 1))], [(8, 8, (8, 1), (1, 1))], [(4, 7, (2, 5), (1, 1)), (4, 13, (2, 1), (1, 1)), (4, 23, (1, 1), (1, 1))], [(2, 11, (2, 3), (1, 1))], [(1, 13, (1, 1), (1, 1))]]

# feature column offsets within a 392-block (reference concat order)
OFF = {k: i * H for i, k in enumerate(
    ["min_v1", "min_i1", "min_v2", "min_i2",
     "max_v1", "max_i1", "max_v2", "max_i2",
     "mean_1", "mean_2",
     "med_v1", "med_i1", "med_v2", "med_i2"])}

# ------------------------------------------------- tile tail-drain workaround
def _patched_drain_and_barrier(self, tick_clock, wait_clock):
    drain_inst = self.nc.sync.drain()
    wait_clock.add_sem_waits(
        drain_inst.ins, tile_mod.ScopedClock({None: tick_clock.global_clock})
    )
    si = drain_inst.ins.sync_info
    waits = list(si.on_wait or [])
    if len(waits) > 1:
        si.on_wait = waits[:1]
        for w in waits[1:]:
            d2 = self.nc.sync.drain()
            si2 = d2.ins.sync_info
            if si2 is None:
                d2.ins.sync_info = mybir.SyncInfo(on_wait=[w], on_update=[])
            else:
                si2.on_wait = [w]
    self.nc.all_engine_barrier()
    assert self.sems is not None
    popped = self.nc._tile_sem_poison_stack.pop()
    assert popped is self._sem_poison
    self.nc.clear_and_free_semaphores(list(self.sems.allocated().values()))
    self.nc.all_engine_barrier()


tile_mod.TileContext._drain_and_barrier = _patched_drain_and_barrier


def _pos2d(base, n1, s1, n2, s2, d):
    """Return access plans for a merged CE group in a position-major
    [p, 28, G] view. Yields ('slc', lo_args, hi_args) per emitted op where
    args describe how to slice. Falls back to splitting when a 2D pattern
    isn't expressible as an einops view."""
    def ok1d(b, n, s):
        return (b, n, s)

    if n1 == 1 or n2 == 1:
        n, s = (n2, s2) if n1 == 1 else (n1, s1)
        yield ("1d", ok1d(base, n, s), ok1d(base + d, n, s))
        return
    # try 2D einops view: requires s2 == 1, s1 | 28, block fits
    def try2d(b):
        if s2 != 1 or 28 % s1 != 0:
            return None
        a0, b0 = b // s1, b % s1
        if b0 + n2 <= s1 and a0 + n1 <= 28 // s1:
            return (a0, b0)
        return None
    lo2, hi2 = try2d(base), try2d(base + d)
    if lo2 is not None and hi2 is not None:
        yield ("2d", (s1, lo2[0], lo2[1], n1, n2), (s1, hi2[0], hi2[1], n1, n2))
        return
    # split along the smaller axis into 1D ops
    if n1 <= n2:
        for u in range(n1):
            b = base + u * s1
            yield ("1d", ok1d(b, n2, s2), ok1d(b + d, n2, s2))
    else:
        for v in range(n2):
            b = base + v * s2
            yield ("1d", ok1d(b, n1, s1), ok1d(b + d, n1, s1))


# ------------------------------------------------------------- bass program
def build_nc(n_tiles: int = N_TILES, debug_features: bool = False):
    nc = bass.Bass()
    t_in = nc.dram_tensor("t", [TILE_IMGS * n_tiles, D], F32,
                          kind="ExternalInput")
    w1 = nc.dram_tensor("w1", [NFEAT, 270], F32, kind="ExternalInput")
    b1 = nc.dram_tensor("b1", [270, 1], F32, kind="ExternalInput")
    w2 = nc.dram_tensor("w2", [270, 90], F32, kind="ExternalInput")
    b2 = nc.dram_tensor("b2", [90, 1], F32, kind="ExternalInput")
    w3 = nc.dram_tensor("w3", [90, 30], F32, kind="ExternalInput")
    b3 = nc.dram_tensor("b3", [30, 1], F32, kind="ExternalInput")
    w4 = nc.dram_tensor("w4", [30, 10], F32, kind="ExternalInput")
    b4 = nc.dram_tensor("b4", [10, 1], F32, kind="ExternalInput")
    idn = nc.dram_tensor("idn", [P, P], F32, kind="ExternalInput")
    cg2 = nc.dram_tensor("cg2", [P, D], F32, kind="ExternalInput")  # col idx
    cg1 = nc.dram_tensor("cg1", [P, D], F32, kind="ExternalInput")  # row idx
    cbias = nc.dram_tensor("cbias", [P, 3], F32, kind="ExternalInput")
    if debug_features:
        y_out = nc.dram_tensor("y", [TILE_IMGS * n_tiles, NFEAT], F32,
                               kind="ExternalOutput")
    else:
        y_out = nc.dram_tensor("y", [TILE_IMGS * n_tiles, 10], F32,
                               kind="ExternalOutput")

    MIN = AluOpType.min
    MAX = AluOpType.max
    ADD = AluOpType.add
    SUB = AluOpType.subtract
    MUL = AluOpType.mult
    MOD = AluOpType.mod
    AXX = mybir.AxisListType.X

    with TileContext(nc) as tc:
        with (
            tc.tile_pool(name="wpool", bufs=1) as wpool,
            tc.tile_pool(name="xpool", bufs=1) as xpool,
            tc.tile_pool(name="kpool", bufs=1) as kpool,
            tc.tile_pool(name="vpool", bufs=2) as vpool,
            tc.tile_pool(name="tspool", bufs=3) as tspool,
            tc.tile_pool(name="fpool", bufs=1) as fpool,
            tc.tile_pool(name="mpool", bufs=1) as mpool,
            tc.tile_pool(name="mxpool", bufs=1) as mxpool,
            tc.tile_pool(name="psT", bufs=2, space="PSUM") as psT,
            tc.tile_pool(name="psL", bufs=2, space="PSUM") as psL,
            tc.tile_pool(name="psS", bufs=2, space="PSUM") as psS,
        ):
            # ---- static weights/consts into SBUF
            w1_t = [wpool.tile([128, 270], F32, name=f"w1_{i}", tag=f"w1_{i}")
                    for i in range(3)]
            w1_t.append(wpool.tile([8, 270], F32, name="w1_3", tag="w1_3"))
            for i in range(3):
                nc.sync.dma_start(w1_t[i][:], w1[128 * i:128 * (i + 1), :])
            nc.sync.dma_start(w1_t[3][:], w1[384:392, :])
            w2_t = [wpool.tile([128, 90], F32, name="w2_0", tag="w2_0"),
                    wpool.tile([128, 90], F32, name="w2_1", tag="w2_1"),
                    wpool.tile([14, 90], F32, name="w2_2", tag="w2_2")]
            nc.sync.dma_start(w2_t[0][:], w2[0:128, :])
            nc.sync.dma_start(w2_t[1][:], w2[128:256, :])
            nc.sync.dma_start(w2_t[2][:], w2[256:270, :])
            w3_t = wpool.tile([90, 30], F32, name="w3", tag="w3")
            nc.sync.dma_start(w3_t[:], w3[:, :])
            w4_t = wpool.tile([30, 10], F32, name="w4", tag="w4")
            nc.sync.dma_start(w4_t[:], w4[:, :])
            b1_t = [wpool.tile([128, 1], F32, name="b1_0", tag="b1_0"),
                    wpool.tile([128, 1], F32, name="b1_1", tag="b1_1"),
                    wpool.tile([14, 1], F32, name="b1_2", tag="b1_2")]
            nc.sync.dma_start(b1_t[0][:], b1[0:128, :])
            nc.sync.dma_start(b1_t[1][:], b1[128:256, :])
            nc.sync.dma_start(b1_t[2][:], b1[256:270, :])
            b2_t = wpool.tile([90, 1], F32, name="b2", tag="b2")
            nc.sync.dma_start(b2_t[:], b2[:, :])
            b3_t = wpool.tile([30, 1], F32, name="b3", tag="b3")
            nc.sync.dma_start(b3_t[:], b3[:, :])
            b4_t = wpool.tile([10, 1], F32, name="b4", tag="b4")
            nc.sync.dma_start(b4_t[:], b4[:, :])
            idn_t = wpool.tile([P, P], F32, name="idn", tag="idn")
            nc.sync.dma_start(idn_t[:], idn[:, :])
            cg2_t = wpool.tile([P, D], F32, name="cg2", tag="cg2")
            nc.sync.dma_start(cg2_t[:], cg2[:, :])
            cg1_t = wpool.tile([P, D], F32, name="cg1", tag="cg1")
            nc.sync.dma_start(cg1_t[:], cg1[:, :])
            cb_t = wpool.tile([P, 3], F32, name="cbias", tag="cbias")
            nc.sync.dma_start(cb_t[:], cbias[:, :])
            bC = cb_t[:, 0:1]        # +C2X
            bNC = cb_t[:, 1:2]       # -C2X
            cg2_b = cg2_t[:].rearrange("p (u d) -> p u d", u=1).broadcast_to(
                [P, PACK, D])
            cg1_b = cg1_t[:].rearrange("p (u d) -> p u d", u=1).broadcast_to(
                [P, PACK, D])
            IDENT = mybir.ActivationFunctionType.Identity

            m1_chunks = [(0, 128), (128, 128), (256, 14)]
            k_chunks = [(0, 128), (128, 128), (256, 128), (384, 8)]

            def prefetch(it):
                """DMA tile it and run the Act-side key integerization."""
                X = xpool.tile([P, FD], F32, name="x", tag="x")
                nc.sync.dma_start(
                    X[:], t_in[TILE_IMGS * it:TILE_IMGS * (it + 1), :]
                    .rearrange("(p i) d -> p (i d)", i=PACK))
                # round() via the RNE +-(2^23+2^19) trick on the fp32 adder.
                # Tt/Tu share the V buffers (bufs=2: tile k uses the slot
                # retired at tile k-2, so this never waits on tile k-1)
                Tt = vpool.tile([P, FD], F32, name="tt", tag="v2")
                Tu = vpool.tile([P, FD], F32, name="tu", tag="v1")
                nc.scalar.activation(Tt[:], X[:], IDENT, bias=bC, scale=QS)
                nc.scalar.activation(Tu[:], Tt[:], IDENT, bias=bNC, scale=1.0)
                return X, Tt, Tu

            nxt = prefetch(0)
            for it in range(n_tiles):
                X, Tt, Tu = nxt

                A1 = kpool.tile([P, FD], F32, name="a1", tag="a1")
                A2 = kpool.tile([P, FD], F32, name="a2", tag="a2")
                Tui = Tu[:].rearrange("p (i d) -> p i d", d=D)
                A1i4 = A1[:].rearrange("p (i d) -> p i d", d=D)
                A2i4 = A2[:].rearrange("p (i d) -> p i d", d=D)
                nc.vector.scalar_tensor_tensor(A1i4, Tui, KS, cg1_b,
                                               op0=MUL, op1=ADD)
                nc.vector.scalar_tensor_tensor(A2i4, Tui, KS, cg2_b,
                                               op0=MUL, op1=ADD)

                V2, V1 = Tt, Tu
                V2x = V2[:].rearrange("p (x g) -> p x g", g=G)
                V1x = V1[:].rearrange("p (x g) -> p x g", g=G)

                # ---- stage 0 (writes every position; no temp/copy needed)
                # axis-2: groups (i,r) stride 28, positions c
                A2x = A2[:].rearrange("p (g x) -> p x g", x=H)
                nc.vector.tensor_tensor(V2x[:, 0:27:2, :], A2x[:, 0:27:2, :],
                                        A2x[:, 1:28:2, :], op=MIN)
                nc.vector.tensor_tensor(V2x[:, 1:28:2, :], A2x[:, 0:27:2, :],
                                        A2x[:, 1:28:2, :], op=MAX)
                # axis-1: groups (i,c), positions r (merged across images)
                A1p = A1[:].rearrange("p (i r c) -> p r i c", r=H, c=H)
                V1p = V1[:].rearrange("p (x i c) -> p x i c", i=PACK, c=H)
                nc.vector.tensor_tensor(V1p[:, 0:27:2, :, :],
                                        A1p[:, 0:27:2, :, :],
                                        A1p[:, 1:28:2, :, :], op=MIN)
                nc.vector.tensor_tensor(V1p[:, 1:28:2, :, :],
                                        A1p[:, 0:27:2, :, :],
                                        A1p[:, 1:28:2, :, :], op=MAX)

                F = fpool.tile([P, PACK * NFEAT], F32, name="feat", tag="feat")
                Ff = F[:].rearrange("p (i f) -> p i f", f=NFEAT)

                # ---- remaining sort stages, interleaved axis-2/axis-1
                def mk_view(Vt, spec, arg):
                    if spec == "1d":
                        b, n, s = arg
                        vx = Vt[:].rearrange("p (x g) -> p x g", g=G)
                        return vx[:, b:b + s * (n - 1) + 1:s, :] if s > 1 \
                            else vx[:, b:b + n, :]
                    s1, a0, b0, n1, n2 = arg
                    vab = Vt[:].rearrange("p (a b g) -> p a b g", b=s1, g=G)
                    return vab[:, a0:a0 + n1, b0:b0 + n2, :]

                def emit_stage(Vt, stage, eng, ts_tag):
                    for (d, st, (n1, s1), (n2, s2)) in stage:
                        for spec, lo_a, hi_a in _pos2d(st, n1, s1, n2, s2, d):
                            lo = mk_view(Vt, spec, lo_a)
                            hi = mk_view(Vt, spec, hi_a)
                            Ts = tspool.tile([P, 11 * G], F32, name="ts",
                                             tag="ts")
                            if spec == "1d":
                                n = lo_a[1]
                                tt = Ts[:].rearrange(
                                    "p (s g) -> p s g", g=G)[:, 0:n, :]
                            else:
                                n1_, n2_ = lo_a[3], lo_a[4]
                                bb = 2 if n2_ <= 2 else 4
                                tt = Ts[:, 0:8 * G].rearrange(
                                    "p (a b g) -> p a b g", b=bb, g=G)[
                                        :, 0:n1_, 0:n2_, :]
                            eng.tensor_tensor(tt, lo, hi, op=MIN)
                            eng.tensor_tensor(hi, lo, hi, op=MAX)
                            nc.scalar.copy(lo, tt)

                for stage in NET28[1:]:
                    emit_stage(V2, stage, nc.vector, "ts2")
                    emit_stage(V1, stage, nc.vector, "ts1")

                if it + 1 < n_tiles:
                    nxt = prefetch(it + 1)

                if pending_softmax is not None:
                    pending_softmax()
                    pending_softmax = None

                # ---- sums (mean features)
                # mean_2: contiguous innermost reduce (full rate)
                Xi = X[:].rearrange("p (i r c) -> p i r c", r=H, c=H)
                o = OFF["mean_2"]
                nc.vector.tensor_reduce(Ff[:, :, o:o + H], Xi, axis=AXX,
                                        op=ADD)
                # mean_1 (column sums): binary add-tree over contiguous row
                # slices (a strided reduce runs at half rate); A1's buffer is
                # dead here and serves as scratch.
                Sv = A1[:].rearrange("p (i r c) -> p i r c", r=H, c=H)
                nc.vector.tensor_tensor(Sv[:, :, 0:14, :], Xi[:, :, 0:14, :],
                                        Xi[:, :, 14:28, :], op=ADD)
                nc.vector.tensor_tensor(Sv[:, :, 0:7, :], Sv[:, :, 0:7, :],
                                        Sv[:, :, 7:14, :], op=ADD)
                nc.vector.tensor_tensor(Sv[:, :, 0:3, :], Sv[:, :, 0:3, :],
                                        Sv[:, :, 3:6, :], op=ADD)
                nc.vector.tensor_tensor(Sv[:, :, 0:1, :], Sv[:, :, 0:1, :],
                                        Sv[:, :, 1:2, :], op=ADD)
                nc.vector.tensor_tensor(Sv[:, :, 0:1, :], Sv[:, :, 0:1, :],
                                        Sv[:, :, 2:3, :], op=ADD)
                o = OFF["mean_1"]
                F1v = Ff[:, :, o:o + H].rearrange("p i (u c) -> p i u c", u=1)
                nc.vector.tensor_tensor(F1v, Sv[:, :, 0:1, :],
                                        Sv[:, :, 6:7, :], op=ADD)

                # ---- feature extraction from key arrays
                # q = rne_int(K/32 - 15.5/32); Fv = q; Fi = K - 32q
                def extract(Vt, pos, vname, iname):
                    K = Vt[:, pos * G:(pos + 1) * G].rearrange(
                        "p (i r) -> p i r", r=H)
                    ov, oi = OFF[vname], OFF[iname]
                    Fv = Ff[:, :, ov:ov + H]
                    Fi = Ff[:, :, oi:oi + H]
                    t1 = mpool.tile([P, G], F32, name="ext1", tag="ext1")
                    t1v = t1[:].rearrange("p (i r) -> p i r", r=H)
                    nc.scalar.activation(t1v, K, IDENT, bias=bC,
                                         scale=1.0 / 32.0)
                    nc.scalar.activation(Fv, t1v, IDENT, bias=bNC, scale=1.0)
                    nc.vector.scalar_tensor_tensor(Fi, Fv, -KS, K,
                                                   op0=MUL, op1=ADD)

                extract(V2, 0, "min_v2", "min_i2")
                extract(V2, 13, "med_v2", "med_i2")
                extract(V2, 27, "max_v2", "max_i2")
                extract(V1, 0, "min_v1", "min_i1")
                extract(V1, 13, "med_v1", "med_i1")
                extract(V1, 27, "max_v1", "max_i1")

                if debug_features:
                    nc.sync.dma_start(
                        y_out[TILE_IMGS * it:TILE_IMGS * (it + 1), :]
                        .rearrange("(p i) f -> p (i f)", i=PACK), F[:])
                    continue

                # ---- MLP (batch 512 = 4 slots x 128 partitions)
                fTs = [mxpool.tile([128, TILE_IMGS], F32, name=f"fts{ci}",
                                  tag=f"fts{ci}") if kc == 128 else
                       mxpool.tile([8, TILE_IMGS], F32, name=f"fts{ci}",
                                  tag=f"fts{ci}")
                       for ci, (k0, kc) in enumerate(k_chunks)]
                for i in range(PACK):
                    for ci, (k0, kc) in enumerate(k_chunks):
                        pt = psT.tile([P, P], F32, name=f"tp{i}_{ci}",
                                      tag="tp")
                        nc.tensor.transpose(
                            pt[0:kc, :], F[:, NFEAT * i + k0:NFEAT * i + k0 + kc],
                            idn_t[:])
                        nc.scalar.copy(fTs[ci][0:kc, 128 * i:128 * (i + 1)],
                                       pt[0:kc, :])

                ex = mpool.tile([10, TILE_IMGS], F32, name="ex", tag="ex")
                for h in range(0, TILE_IMGS, 512):
                    hs = slice(h, h + 512)
                    a1 = []
                    for mi, (m0, mc) in enumerate(m1_chunks):
                        ps = psL.tile([P, 512], F32, name=f"l1_{m0}",
                                      tag="l1")[0:mc, :]
                        for ci, (k0, kc) in enumerate(k_chunks):
                            nc.tensor.matmul(ps[:], w1_t[ci][0:kc, m0:m0 + mc],
                                             fTs[ci][0:kc, hs],
                                             start=(ci == 0), stop=(ci == 3))
                        sb = mpool.tile([P, 512], F32, name=f"a1_{m0}",
                                        tag=f"a1_{m0}")[0:mc, :]
                        nc.scalar.activation(sb, ps,
                                             mybir.ActivationFunctionType.Relu,
                                             bias=b1_t[mi][0:mc, :], scale=1.0)
                        a1.append(sb)

                    ps2 = psS.tile([P, 512], F32, name="l2",
                                   tag="l2")[0:90, :]
                    for ci, (k0, kc) in enumerate(m1_chunks):
                        nc.tensor.matmul(ps2[:], w2_t[ci][0:kc, :],
                                         a1[ci][0:kc, :] if kc != 128 else a1[ci],
                                         start=(ci == 0), stop=(ci == 2))
                    a2t = mpool.tile([90, 512], F32, name="a2", tag="a2")
                    nc.scalar.activation(a2t[:], ps2,
                                         mybir.ActivationFunctionType.Relu,
                                         bias=b2_t[:], scale=1.0)

                    ps3 = psS.tile([P, 512], F32, name="l3",
                                   tag="l3")[0:30, :]
                    nc.tensor.matmul(ps3[:], w3_t[:], a2t[:], start=True,
                                     stop=True)
                    a3t = mpool.tile([30, 512], F32, name="a3", tag="a3")
                    nc.scalar.activation(a3t[:], ps3,
                                         mybir.ActivationFunctionType.Relu,
                                         bias=b3_t[:], scale=1.0)

                    ps4 = psS.tile([P, 512], F32, name="l4",
                                   tag="l2")[0:10, :]
                    nc.tensor.matmul(ps4[:], w4_t[:], a3t[:], start=True,
                                     stop=True)
                    nc.scalar.activation(ex[:, hs], ps4,
                                         mybir.ActivationFunctionType.Exp,
                                         bias=b4_t[:], scale=1.0)

                ext = mpool.tile([P, PACK * 10], F32, name="ext", tag="ext")
                for i in range(PACK):
                    pst = psT.tile([P, P], F32, name=f"sm{i}", tag="tp")
                    nc.tensor.transpose(pst[:, 0:10],
                                        ex[:, 128 * i:128 * (i + 1)],
                                        idn_t[0:10, 0:10])
                    nc.scalar.copy(ext[:, 10 * i:10 * (i + 1)], pst[:, 0:10])
                def softmax_fin(it=it, ext=ext):
                    exi = ext[:].rearrange("p (i u) -> p i u", u=10)
                    sums = mpool.tile([P, PACK], F32, name="sums", tag="sums")
                    nc.vector.tensor_reduce(sums[:], exi, axis=AXX, op=ADD)
                    rcp = mpool.tile([P, PACK], F32, name="rcp", tag="rcp")
                    nc.vector.reciprocal(rcp[:], sums[:])
                    yt = mpool.tile([P, PACK * 10], F32, name="yt", tag="yt")
                    rcb = rcp[:].rearrange("p (i u) -> p i u",
                                           u=1).broadcast_to([P, PACK, 10])
                    nc.vector.tensor_tensor(
                        yt[:].rearrange("p (i u) -> p i u", u=10),
                        exi, rcb, op=MUL)
                    nc.sync.dma_start(
                        y_out[TILE_IMGS * it:TILE_IMGS * (it + 1), :]
                        .rearrange("(p i) u -> p i u", i=PACK),
                        yt[:].rearrange("p (i u) -> p i u", u=10))
                pending_softmax = softmax_fin

            if pending_softmax is not None:
                pending_softmax()

    _split_excess_waits(nc)
    return nc


MAX_WAITS = 1


def _split_excess_waits(nc):
    """Walrus in this container rejects instructions with >MAX_WAITS sem
    waits; hoist the excess onto NoOp carriers inserted just before."""
    import bass_rust
    ctr = [0]
    for f in nc.m.functions:
        for blk in f.blocks:
            insts = list(blk.instructions)
            out = []
            changed = False
            for inst in insts:
                si = inst.sync_info
                waits = list(si.on_wait) if (si and si.on_wait) else []
                if len(waits) > MAX_WAITS:
                    changed = True
                    excess = waits[:-MAX_WAITS]
                    si.on_wait = waits[-MAX_WAITS:]
                    for k in range(0, len(excess), MAX_WAITS):
                        nop = bass_rust.InstNoOp(
                            name=f"WSPLIT-{ctr[0]}", ins=[], outs=[])
                        ctr[0] += 1
                        nop.engine = inst.engine
                        nop.sync_info = mybir.SyncInfo(
                            on_wait=excess[k:k + MAX_WAITS], on_update=[])
                        out.append(nop)
                out.append(inst)
            if changed:
                blk.instructions = out


# ------------------------------------------------------------- numpy driver
def _prep_weights(W1, b1, W2, b2, W3, b3, W4, b4):
    """Fold per-feature affine corrections into W1/b1; return transposed
    weight matrices plus constant tiles."""
    scale = np.ones(NFEAT, np.float64)
    offset = np.zeros(NFEAT, np.float64)
    for name in ("min_v1", "min_v2", "max_v1", "max_v2",
                 "med_v1", "med_v2"):
        o = OFF[name]
        scale[o:o + H] = 1.0 / QS
    for name in ("mean_1", "mean_2"):
        o = OFF[name]
        scale[o:o + H] = 1.0 / H
    for name in ("min_i1", "min_i2", "max_i1", "max_i2",
                 "med_i1", "med_i2"):
        o = OFF[name]
        offset[o:o + H] = 15.5
    W1_eff = W1.astype(np.float64) * scale[None, :]
    b1_eff = b1.astype(np.float64) + W1.astype(np.float64) @ offset
    c2 = np.tile(np.arange(H, dtype=np.float32) - 15.5, H)          # col idx
    c1 = np.repeat(np.arange(H, dtype=np.float32) - 15.5, H)         # row idx
    return {
        "w1": np.ascontiguousarray(W1_eff.T.astype(np.float32)),
        "b1": b1_eff.astype(np.float32).reshape(-1, 1),
        "w2": np.ascontiguousarray(W2.T.astype(np.float32)),
        "b2": b2.reshape(-1, 1).astype(np.float32),
        "w3": np.ascontiguousarray(W3.T.astype(np.float32)),
        "b3": b3.reshape(-1, 1).astype(np.float32),
        "w4": np.ascontiguousarray(W4.T.astype(np.float32)),
        "b4": b4.reshape(-1, 1).astype(np.float32),
        "idn": np.eye(P, dtype=np.float32),
        "cg2": np.broadcast_to(c2[None, :], (P, D)).copy(),
        "cg1": np.broadcast_to(c1[None, :], (P, D)).copy(),
        "cbias": np.broadcast_to(np.array(
            [C2X, -C2X, 0.0], np.float32)[None, :], (P, 3)).copy(),
    }


_NC_CACHE = {}


def _get_nc(n_tiles, debug_features, **kw):
    key = (n_tiles, debug_features, tuple(sorted(kw.items())))
    if key not in _NC_CACHE:
        _NC_CACHE[key] = build_nc(n_tiles, debug_features, **kw)
    return _NC_CACHE[key]


def run(t, weights, n_tiles=N_TILES, debug_features=False, trace=False, **kw):
    nc = _get_nc(n_tiles, debug_features, **kw)
    rows = TILE_IMGS * n_tiles
    in_maps = []
    for c in range(N_CORES):
        m = {"t": np.ascontiguousarray(t[c * B_CORE:c * B_CORE + rows])}
        m.update(weights)
        in_maps.append(m)
    res = run_bass_kernel_spmd(nc, in_maps, core_ids=list(range(N_CORES)),
                               trace=trace)
    outs = [r["y"] for r in res.results]
    return outs, res


def kernel(t, W1, b1, W2, b2, W3, b3, W4, b4):
    weights = _prep_weights(W1, b1, W2, b2, W3, b3, W4, b4)
    outs, _ = run(t, weights)
    y = np.concatenate(outs, axis=0)
    return np.ascontiguousarray(y.astype(np.float32))


# revision 23
# speedup vs baseline: 1.3175x; 1.0982x over previous
"""Trainium2 Bass kernel for nn_CNNModel_42064909697048.

Per-image row/col statistics (min/argmin/max/argmax/mean/median/argmedian
over both axes of each 28x28 image) -> 392 features -> 4-layer MLP ->
softmax, data-parallel over 8 NeuronCores.

Approach: values are packed into integer-exact fp32 keys
    key = 32*trunc(x*65536) + local_index
so a single min/max/rank-select on keys yields both the value and its
argindex (ties break toward the smaller index, matching numpy/torch).
Min, lower-median (rank 13) and max are produced simultaneously by one
Batcher odd-even sorting network pruned to outputs {0,13,27} (133
compare-exchanges), vectorized across 4 images x 28 groups per partition
in a position-major layout. Work is split across the Vector (axis-2 sort,
key build), GpSimd (axis-1 sort, sums) and Scalar (compare-exchange
copy-backs, activations) engines; the MLP runs on the tensor engine with
batch-512 matmuls. Index/scale corrections are folded into W1/b1.

Self-contained: hardcodes shapes/sharding; no sibling imports.
"""

import numpy as np

import concourse.bass as bass
import concourse.mybir as mybir
import concourse.tile as tile_mod
from concourse.tile import TileContext
from concourse.bass_utils import run_bass_kernel_spmd
from concourse.alu_op_type import AluOpType

# ---------------------------------------------------------------- constants
B_TOTAL = 131072
N_CORES = 8
B_CORE = B_TOTAL // N_CORES          # 16384
H = 28
D = 784
P = 128
PACK = 8                             # images per partition
TILE_IMGS = P * PACK                 # 512
N_TILES = B_CORE // TILE_IMGS        # 32
G = PACK * H                         # sort groups per partition = 112
FD = PACK * D                        # free dim of an image tile = 3136
NFEAT = 392
QS = 32768.0                         # value quantization scale (2^15)
KS = 32.0                            # index slots per quantum
C2X = float(2**23 + 2**19)           # RNE integerization bias (covers +-2^19)
F32 = mybir.dt.float32

# Batcher odd-even mergesort net for 28, pruned to outputs {0,13,27};
# stages of merged groups (d, start, (n1,s1), (n2,s2)):
# lo positions = {start + u*s1 + v*s2}, hi = lo + d.
NET28 = [[(1, 0, (14, 2), (1, 1))], [(2, 0, (7, 4), (2, 1))], [(1, 1, (7, 4), (1, 1)), (4, 0, (3, 8), (2, 3))], [(4, 1, (3, 8), (2, 1)), (8, 0, (2, 7), (1, 1)), (8, 16, (1, 1), (1, 1)), (1, 25, (1, 1), (1, 1))], [(2, 2, (3, 8), (2, 1)), (16, 0, (1, 1), (1, 1))], [(1, 1, (3, 8), (3, 2))], [(8, 1, (6, 1), (1, 1)), (8, 17, (3, 1), (1, 1)), (4, 20, (1, 1), (1, 1))], [(4, 4, (2, 17), (3, 1)), (4, 7, (1, 1), (1, 1)), (2, 18, (1, 1), (1, 1))], [(2, 2, (3, 4), (2, 1)), (2, 19, (2, 3), (1, 1)), (2, 23, (1, 1), (1, 1)), (1, 17, (1, 1), (1, 1))], [(1, 1, (2, 18), (4, 2)), (1, 9, (3, 2), (1, 1))], [(16, 1, (11, 1), (1, 1))], [(8, 8, (8, 1), (1, 1))], [(4, 7, (2, 5), (1, 1)), (4, 13, (2, 1), (1, 1)), (4, 23, (1, 1), (1, 1))], [(2, 11, (2, 3), (1, 1))], [(1, 13, (1, 1), (1, 1))]]

# feature column offsets within a 392-block (reference concat order)
OFF = {k: i * H for i, k in enumerate(
    ["min_v1", "min_i1", "min_v2", "min_i2",
     "max_v1", "max_i1", "max_v2", "max_i2",
     "mean_1", "mean_2",
     "med_v1", "med_i1", "med_v2", "med_i2"])}

# ------------------------------------------------- tile tail-drain workaround
def _patched_drain_and_barrier(self, tick_clock, wait_clock):
    drain_inst = self.nc.sync.drain()
    wait_clock.add_sem_waits(
        drain_inst.ins, tile_mod.ScopedClock({None: tick_clock.global_clock})
    )
    si = drain_inst.ins.sync_info
    waits = list(si.on_wait or [])
    if len(waits) > 1:
        si.on_wait = waits[:1]
        for w in waits[1:]:
            d2 = self.nc.sync.drain()
            si2 = d2.ins.sync_info
            if si2 is None:
                d2.ins.sync_info = mybir.SyncInfo(on_wait=[w], on_update=[])
            else:
                si2.on_wait = [w]
    self.nc.all_engine_barrier()
    assert self.sems is not None
    popped = self.nc._tile_sem_poison_stack.pop()
    assert popped is self._sem_poison
    self.nc.clear_and_free_semaphores(list(self.sems.allocated().values()))
    self.nc.all_engine_barrier()


tile_mod.TileContext._drain_and_barrier = _patched_drain_and_barrier


def _pos2d(base, n1, s1, n2, s2, d):
    """Return access plans for a merged CE group in a position-major
    [p, 28, G] view. Yields ('slc', lo_args, hi_args) per emitted op where
    args describe how to slice. Falls back to splitting when a 2D pattern
    isn't expressible as an einops view."""
    def ok1d(b, n, s):
        return (b, n, s)

    if n1 == 1 or n2 == 1:
        n, s = (n2, s2) if n1 == 1 else (n1, s1)
        yield ("1d", ok1d(base, n, s), ok1d(base + d, n, s))
        return
    # try 2D einops view: requires s2 == 1, s1 | 28, block fits
    def try2d(b):
        if s2 != 1 or 28 % s1 != 0:
            return None
        a0, b0 = b // s1, b % s1
        if b0 + n2 <= s1 and a0 + n1 <= 28 // s1:
            return (a0, b0)
        return None
    lo2, hi2 = try2d(base), try2d(base + d)
    if lo2 is not None and hi2 is not None:
        yield ("2d", (s1, lo2[0], lo2[1], n1, n2), (s1, hi2[0], hi2[1], n1, n2))
        return
    # split along the smaller axis into 1D ops
    if n1 <= n2:
        for u in range(n1):
            b = base + u * s1
            yield ("1d", ok1d(b, n2, s2), ok1d(b + d, n2, s2))
    else:
        for v in range(n2):
            b = base + v * s2
            yield ("1d", ok1d(b, n1, s1), ok1d(b + d, n1, s1))


# ------------------------------------------------------------- bass program
def build_nc(n_tiles: int = N_TILES, debug_features: bool = False):
    nc = bass.Bass()
    t_in = nc.dram_tensor("t", [TILE_IMGS * n_tiles, D], F32,
                          kind="ExternalInput")
    w1 = nc.dram_tensor("w1", [NFEAT, 270], F32, kind="ExternalInput")
    b1 = nc.dram_tensor("b1", [270, 1], F32, kind="ExternalInput")
    w2 = nc.dram_tensor("w2", [270, 90], F32, kind="ExternalInput")
    b2 = nc.dram_tensor("b2", [90, 1], F32, kind="ExternalInput")
    w3 = nc.dram_tensor("w3", [90, 30], F32, kind="ExternalInput")
    b3 = nc.dram_tensor("b3", [30, 1], F32, kind="ExternalInput")
    w4 = nc.dram_tensor("w4", [30, 10], F32, kind="ExternalInput")
    b4 = nc.dram_tensor("b4", [10, 1], F32, kind="ExternalInput")
    idn = nc.dram_tensor("idn", [P, P], F32, kind="ExternalInput")
    cg2 = nc.dram_tensor("cg2", [P, D], F32, kind="ExternalInput")  # col idx
    cg1 = nc.dram_tensor("cg1", [P, D], F32, kind="ExternalInput")  # row idx
    cbias = nc.dram_tensor("cbias", [P, 3], F32, kind="ExternalInput")
    if debug_features:
        y_out = nc.dram_tensor("y", [TILE_IMGS * n_tiles, NFEAT], F32,
                               kind="ExternalOutput")
    else:
        y_out = nc.dram_tensor("y", [TILE_IMGS * n_tiles, 10], F32,
                               kind="ExternalOutput")

    MIN = AluOpType.min
    MAX = AluOpType.max
    ADD = AluOpType.add
    SUB = AluOpType.subtract
    MUL = AluOpType.mult
    MOD = AluOpType.mod
    AXX = mybir.AxisListType.X

    with TileContext(nc) as tc:
        with (
            tc.tile_pool(name="wpool", bufs=1) as wpool,
            tc.tile_pool(name="xpool", bufs=1) as xpool,
            tc.tile_pool(name="kpool", bufs=1) as kpool,
            tc.tile_pool(name="vpool", bufs=2) as vpool,
            tc.tile_pool(name="tspool", bufs=4) as tspool,
            tc.tile_pool(name="fpool", bufs=1) as fpool,
            tc.tile_pool(name="mpool", bufs=1) as mpool,
            tc.tile_pool(name="mxpool", bufs=1) as mxpool,
            tc.tile_pool(name="psT", bufs=2, space="PSUM") as psT,
            tc.tile_pool(name="psL", bufs=2, space="PSUM") as psL,
            tc.tile_pool(name="psS", bufs=2, space="PSUM") as psS,
        ):
            # ---- static weights/consts into SBUF
            w1_t = [wpool.tile([128, 270], F32, name=f"w1_{i}", tag=f"w1_{i}")
                    for i in range(3)]
            w1_t.append(wpool.tile([8, 270], F32, name="w1_3", tag="w1_3"))
            for i in range(3):
                nc.sync.dma_start(w1_t[i][:], w1[128 * i:128 * (i + 1), :])
            nc.sync.dma_start(w1_t[3][:], w1[384:392, :])
            w2_t = [wpool.tile([128, 90], F32, name="w2_0", tag="w2_0"),
                    wpool.tile([128, 90], F32, name="w2_1", tag="w2_1"),
                    wpool.tile([14, 90], F32, name="w2_2", tag="w2_2")]
            nc.sync.dma_start(w2_t[0][:], w2[0:128, :])
            nc.sync.dma_start(w2_t[1][:], w2[128:256, :])
            nc.sync.dma_start(w2_t[2][:], w2[256:270, :])
            w3_t = wpool.tile([90, 30], F32, name="w3", tag="w3")
            nc.sync.dma_start(w3_t[:], w3[:, :])
            w4_t = wpool.tile([30, 10], F32, name="w4", tag="w4")
            nc.sync.dma_start(w4_t[:], w4[:, :])
            b1_t = [wpool.tile([128, 1], F32, name="b1_0", tag="b1_0"),
                    wpool.tile([128, 1], F32, name="b1_1", tag="b1_1"),
                    wpool.tile([14, 1], F32, name="b1_2", tag="b1_2")]
            nc.sync.dma_start(b1_t[0][:], b1[0:128, :])
            nc.sync.dma_start(b1_t[1][:], b1[128:256, :])
            nc.sync.dma_start(b1_t[2][:], b1[256:270, :])
            b2_t = wpool.tile([90, 1], F32, name="b2", tag="b2")
            nc.sync.dma_start(b2_t[:], b2[:, :])
            b3_t = wpool.tile([30, 1], F32, name="b3", tag="b3")
            nc.sync.dma_start(b3_t[:], b3[:, :])
            b4_t = wpool.tile([10, 1], F32, name="b4", tag="b4")
            nc.sync.dma_start(b4_t[:], b4[:, :])
            idn_t = wpool.tile([P, P], F32, name="idn", tag="idn")
            nc.sync.dma_start(idn_t[:], idn[:, :])
            cg2_t = wpool.tile([P, D], F32, name="cg2", tag="cg2")
            nc.sync.dma_start(cg2_t[:], cg2[:, :])
            cg1_t = wpool.tile([P, D], F32, name="cg1", tag="cg1")
            nc.sync.dma_start(cg1_t[:], cg1[:, :])
            cb_t = wpool.tile([P, 3], F32, name="cbias", tag="cbias")
            nc.sync.dma_start(cb_t[:], cbias[:, :])
            bC = cb_t[:, 0:1]        # +C2X
            bNC = cb_t[:, 1:2]       # -C2X
            cg2_b = cg2_t[:].rearrange("p (u d) -> p u d", u=1).broadcast_to(
                [P, PACK, D])
            cg1_b = cg1_t[:].rearrange("p (u d) -> p u d", u=1).broadcast_to(
                [P, PACK, D])
            IDENT = mybir.ActivationFunctionType.Identity

            m1_chunks = [(0, 128), (128, 128), (256, 14)]
            k_chunks = [(0, 128), (128, 128), (256, 128), (384, 8)]

            def prefetch(it):
                """DMA tile it and run the Act-side key integerization."""
                X = xpool.tile([P, FD], F32, name="x", tag="x")
                nc.sync.dma_start(
                    X[:], t_in[TILE_IMGS * it:TILE_IMGS * (it + 1), :]
                    .rearrange("(p i) d -> p (i d)", i=PACK))
                # round() via the RNE +-(2^23+2^19) trick on the fp32 adder.
                # Tt/Tu share the V buffers (bufs=2: tile k uses the slot
                # retired at tile k-2, so this never waits on tile k-1)
                Tt = vpool.tile([P, FD], F32, name="tt", tag="v2")
                Tu = vpool.tile([P, FD], F32, name="tu", tag="v1")
                nc.scalar.activation(Tt[:], X[:], IDENT, bias=bC, scale=QS)
                nc.scalar.activation(Tu[:], Tt[:], IDENT, bias=bNC, scale=1.0)
                return X, Tt, Tu

            nxt = prefetch(0)
            for it in range(n_tiles):
                X, Tt, Tu = nxt

                A1 = kpool.tile([P, FD], F32, name="a1", tag="a1")
                A2 = kpool.tile([P, FD], F32, name="a2", tag="a2")
                Tui = Tu[:].rearrange("p (i d) -> p i d", d=D)
                A1i4 = A1[:].rearrange("p (i d) -> p i d", d=D)
                A2i4 = A2[:].rearrange("p (i d) -> p i d", d=D)
                nc.vector.scalar_tensor_tensor(A1i4, Tui, KS, cg1_b,
                                               op0=MUL, op1=ADD)
                nc.vector.scalar_tensor_tensor(A2i4, Tui, KS, cg2_b,
                                               op0=MUL, op1=ADD)

                V2, V1 = Tt, Tu
                V2x = V2[:].rearrange("p (x g) -> p x g", g=G)
                V1x = V1[:].rearrange("p (x g) -> p x g", g=G)

                # ---- stage 0 (writes every position; no temp/copy needed)
                # axis-2: groups (i,r) stride 28, positions c
                A2x = A2[:].rearrange("p (g x) -> p x g", x=H)
                nc.vector.tensor_tensor(V2x[:, 0:27:2, :], A2x[:, 0:27:2, :],
                                        A2x[:, 1:28:2, :], op=MIN)
                nc.vector.tensor_tensor(V2x[:, 1:28:2, :], A2x[:, 0:27:2, :],
                                        A2x[:, 1:28:2, :], op=MAX)
                # axis-1: groups (i,c), positions r (merged across images)
                A1p = A1[:].rearrange("p (i r c) -> p r i c", r=H, c=H)
                V1p = V1[:].rearrange("p (x i c) -> p x i c", i=PACK, c=H)
                nc.vector.tensor_tensor(V1p[:, 0:27:2, :, :],
                                        A1p[:, 0:27:2, :, :],
                                        A1p[:, 1:28:2, :, :], op=MIN)
                nc.vector.tensor_tensor(V1p[:, 1:28:2, :, :],
                                        A1p[:, 0:27:2, :, :],
                                        A1p[:, 1:28:2, :, :], op=MAX)

                F = fpool.tile([P, PACK * NFEAT], F32, name="feat", tag="feat")
                Ff = F[:].rearrange("p (i f) -> p i f", f=NFEAT)

                # ---- remaining sort stages, interleaved axis-2/axis-1
                def mk_view(Vt, spec, arg):
                    if spec == "1d":
                        b, n, s = arg
                        vx = Vt[:].rearrange("p (x g) -> p x g", g=G)
                        return vx[:, b:b + s * (n - 1) + 1:s, :] if s > 1 \
                            else vx[:, b:b + n, :]
                    s1, a0, b0, n1, n2 = arg
                    vab = Vt[:].rearrange("p (a b g) -> p a b g", b=s1, g=G)
                    return vab[:, a0:a0 + n1, b0:b0 + n2, :]

                def emit_stage(Vt, stage, eng, ts_tag):
                    for (d, st, (n1, s1), (n2, s2)) in stage:
                        for spec, lo_a, hi_a in _pos2d(st, n1, s1, n2, s2, d):
                            lo = mk_view(Vt, spec, lo_a)
                            hi = mk_view(Vt, spec, hi_a)
                            Ts = tspool.tile([P, 8 * G], F32, name="ts",
                                             tag="ts")
                            if spec == "1d":
                                n = lo_a[1]
                                tt = Ts[:].rearrange(
                                    "p (s g) -> p s g", g=G)[:, 0:n, :]
                            else:
                                n1_, n2_ = lo_a[3], lo_a[4]
                                bb = 2 if n2_ <= 2 else 4
                                tt = Ts[:, 0:8 * G].rearrange(
                                    "p (a b g) -> p a b g", b=bb, g=G)[
                                        :, 0:n1_, 0:n2_, :]
                            eng.tensor_tensor(tt, lo, hi, op=MIN)
                            eng.tensor_tensor(hi, lo, hi, op=MAX)
                            nc.scalar.copy(lo, tt)

                for stage in NET28[1:]:
                    emit_stage(V2, stage, nc.vector, "ts2")
                    emit_stage(V1, stage, nc.vector, "ts1")

                if it + 1 < n_tiles:
                    nxt = prefetch(it + 1)

                if pending_softmax is not None:
                    pending_softmax()
                    pending_softmax = None

                # ---- sums (mean features)
                # mean_2: contiguous innermost reduce (full rate)
                Xi = X[:].rearrange("p (i r c) -> p i r c", r=H, c=H)
                o = OFF["mean_2"]
                nc.vector.tensor_reduce(Ff[:, :, o:o + H], Xi, axis=AXX,
                                        op=ADD)
                # mean_1 (column sums): binary add-tree over contiguous row
                # slices (a strided reduce runs at half rate); A1's buffer is
                # dead here and serves as scratch.
                Sv = A1[:].rearrange("p (i r c) -> p i r c", r=H, c=H)
                nc.vector.tensor_tensor(Sv[:, :, 0:14, :], Xi[:, :, 0:14, :],
                                        Xi[:, :, 14:28, :], op=ADD)
                nc.vector.tensor_tensor(Sv[:, :, 0:7, :], Sv[:, :, 0:7, :],
                                        Sv[:, :, 7:14, :], op=ADD)
                nc.vector.tensor_tensor(Sv[:, :, 0:3, :], Sv[:, :, 0:3, :],
                                        Sv[:, :, 3:6, :], op=ADD)
                nc.vector.tensor_tensor(Sv[:, :, 0:1, :], Sv[:, :, 0:1, :],
                                        Sv[:, :, 1:2, :], op=ADD)
                nc.vector.tensor_tensor(Sv[:, :, 0:1, :], Sv[:, :, 0:1, :],
                                        Sv[:, :, 2:3, :], op=ADD)
                o = OFF["mean_1"]
                F1v = Ff[:, :, o:o + H].rearrange("p i (u c) -> p i u c", u=1)
                nc.vector.tensor_tensor(F1v, Sv[:, :, 0:1, :],
                                        Sv[:, :, 6:7, :], op=ADD)

                # ---- feature extraction from key arrays
                # q = rne_int(K/32 - 15.5/32); Fv = q; Fi = K - 32q
                def extract(Vt, pos, vname, iname):
                    K = Vt[:, pos * G:(pos + 1) * G].rearrange(
                        "p (i r) -> p i r", r=H)
                    ov, oi = OFF[vname], OFF[iname]
                    Fv = Ff[:, :, ov:ov + H]
                    Fi = Ff[:, :, oi:oi + H]
                    t1 = mpool.tile([P, G], F32, name="ext1", tag="ext1")
                    t1v = t1[:].rearrange("p (i r) -> p i r", r=H)
                    nc.scalar.activation(t1v, K, IDENT, bias=bC,
                                         scale=1.0 / 32.0)
                    nc.scalar.activation(Fv, t1v, IDENT, bias=bNC, scale=1.0)
                    nc.vector.scalar_tensor_tensor(Fi, Fv, -KS, K,
                                                   op0=MUL, op1=ADD)

                extract(V2, 0, "min_v2", "min_i2")
                extract(V2, 13, "med_v2", "med_i2")
                extract(V2, 27, "max_v2", "max_i2")
                extract(V1, 0, "min_v1", "min_i1")
                extract(V1, 13, "med_v1", "med_i1")
                extract(V1, 27, "max_v1", "max_i1")

                if debug_features:
                    nc.sync.dma_start(
                        y_out[TILE_IMGS * it:TILE_IMGS * (it + 1), :]
                        .rearrange("(p i) f -> p (i f)", i=PACK), F[:])
                    continue

                # ---- MLP (batch 512 = 4 slots x 128 partitions)
                fTs = [mxpool.tile([128, TILE_IMGS], F32, name=f"fts{ci}",
                                  tag=f"fts{ci}") if kc == 128 else
                       mxpool.tile([8, TILE_IMGS], F32, name=f"fts{ci}",
                                  tag=f"fts{ci}")
                       for ci, (k0, kc) in enumerate(k_chunks)]
                for i in range(PACK):
                    for ci, (k0, kc) in enumerate(k_chunks):
                        pt = psT.tile([P, P], F32, name=f"tp{i}_{ci}",
                                      tag="tp")
                        nc.tensor.transpose(
                            pt[0:kc, :], F[:, NFEAT * i + k0:NFEAT * i + k0 + kc],
                            idn_t[:])
                        nc.scalar.copy(fTs[ci][0:kc, 128 * i:128 * (i + 1)],
                                       pt[0:kc, :])

                ex = mpool.tile([10, TILE_IMGS], F32, name="ex", tag="ex")
                for h in range(0, TILE_IMGS, 512):
                    hs = slice(h, h + 512)
                    a1 = []
                    for mi, (m0, mc) in enumerate(m1_chunks):
                        ps = psL.tile([P, 512], F32, name=f"l1_{m0}",
                                      tag="l1")[0:mc, :]
                        for ci, (k0, kc) in enumerate(k_chunks):
                            nc.tensor.matmul(ps[:], w1_t[ci][0:kc, m0:m0 + mc],
                                             fTs[ci][0:kc, hs],
                                             start=(ci == 0), stop=(ci == 3))
                        sb = mpool.tile([P, 512], F32, name=f"a1_{m0}",
                                        tag=f"a1_{m0}")[0:mc, :]
                        nc.scalar.activation(sb, ps,
                                             mybir.ActivationFunctionType.Relu,
                                             bias=b1_t[mi][0:mc, :], scale=1.0)
                        a1.append(sb)

                    ps2 = psS.tile([P, 512], F32, name="l2",
                                   tag="l2")[0:90, :]
                    for ci, (k0, kc) in enumerate(m1_chunks):
                        nc.tensor.matmul(ps2[:], w2_t[ci][0:kc, :],
                                         a1[ci][0:kc, :] if kc != 128 else a1[ci],
                                         start=(ci == 0), stop=(ci == 2))
                    a2t = mpool.tile([90, 512], F32, name="a2", tag="a2")
                    nc.scalar.activation(a2t[:], ps2,
                                         mybir.ActivationFunctionType.Relu,
                                         bias=b2_t[:], scale=1.0)

                    ps3 = psS.tile([P, 512], F32, name="l3",
                                   tag="l3")[0:30, :]
                    nc.tensor.matmul(ps3[:], w3_t[:], a2t[:], start=True,
                                     stop=True)
                    a3t = mpool.tile([30, 512], F32, name="a3", tag="a3")
                    nc.scalar.activation(a3t[:], ps3,
                                         mybir.ActivationFunctionType.Relu,
                                         bias=b3_t[:], scale=1.0)

                    ps4 = psS.tile([P, 512], F32, name="l4",
                                   tag="l2")[0:10, :]
                    nc.tensor.matmul(ps4[:], w4_t[:], a3t[:], start=True,
                                     stop=True)
                    nc.scalar.activation(ex[:, hs], ps4,
                                         mybir.ActivationFunctionType.Exp,
                                         bias=b4_t[:], scale=1.0)

                ext = mpool.tile([P, PACK * 10], F32, name="ext", tag="ext")
                for i in range(PACK):
                    pst = psT.tile([P, P], F32, name=f"sm{i}", tag="tp")
                    nc.tensor.transpose(pst[:, 0:10],
                                        ex[:, 128 * i:128 * (i + 1)],
                                        idn_t[0:10, 0:10])
                    nc.scalar.copy(ext[:, 10 * i:10 * (i + 1)], pst[:, 0:10])
                def softmax_fin(it=it, ext=ext):
                    exi = ext[:].rearrange("p (i u) -> p i u", u=10)
                    sums = mpool.tile([P, PACK], F32, name="sums", tag="sums")
                    nc.vector.tensor_reduce(sums[:], exi, axis=AXX, op=ADD)
                    rcp = mpool.tile([P, PACK], F32, name="rcp", tag="rcp")
                    nc.vector.reciprocal(rcp[:], sums[:])
                    yt = mpool.tile([P, PACK * 10], F32, name="yt", tag="yt")
                    rcb = rcp[:].rearrange("p (i u) -> p i u",
                                           u=1).broadcast_to([P, PACK, 10])
                    nc.vector.tensor_tensor(
                        yt[:].rearrange("p (i u) -> p i u", u=10),
                        exi, rcb, op=MUL)
                    nc.sync.dma_start(
                        y_out[TILE_IMGS * it:TILE_IMGS * (it + 1), :]
                        .rearrange("(p i) u -> p i u", i=PACK),
                        yt[:].rearrange("p (i u) -> p i u", u=10))
                pending_softmax = softmax_fin

            if pending_softmax is not None:
                pending_softmax()

    _split_excess_waits(nc)
    return nc


MAX_WAITS = 1


def _split_excess_waits(nc):
    """Walrus in this container rejects instructions with >MAX_WAITS sem
    waits; hoist the excess onto NoOp carriers inserted just before."""
    import bass_rust
    ctr = [0]
    for f in nc.m.functions:
        for blk in f.blocks:
            insts = list(blk.instructions)
            out = []
            changed = False
            for inst in insts:
                si = inst.sync_info
                waits = list(si.on_wait) if (si and si.on_wait) else []
                if len(waits) > MAX_WAITS:
                    changed = True
                    excess = waits[:-MAX_WAITS]
                    si.on_wait = waits[-MAX_WAITS:]
                    for k in range(0, len(excess), MAX_WAITS):
                        nop = bass_rust.InstNoOp(
                            name=f"WSPLIT-{ctr[0]}", ins=[], outs=[])
                        ctr[0] += 1
                        nop.engine = inst.engine
                        nop.sync_info = mybir.SyncInfo(
                            on_wait=excess[k:k + MAX_WAITS], on_update=[])
                        out.append(nop)
                out.append(inst)
            if changed:
                blk.instructions = out


# ------------------------------------------------------------- numpy driver
def _prep_weights(W1, b1, W2, b2, W3, b3, W4, b4):
    """Fold per-feature affine corrections into W1/b1; return transposed
    weight matrices plus constant tiles."""
    scale = np.ones(NFEAT, np.float64)
    offset = np.zeros(NFEAT, np.float64)
    for name in ("min_v1", "min_v2", "max_v1", "max_v2",
                 "med_v1", "med_v2"):
        o = OFF[name]
        scale[o:o + H] = 1.0 / QS
    for name in ("mean_1", "mean_2"):
        o = OFF[name]
        scale[o:o + H] = 1.0 / H
    for name in ("min_i1", "min_i2", "max_i1", "max_i2",
                 "med_i1", "med_i2"):
        o = OFF[name]
        offset[o:o + H] = 15.5
    W1_eff = W1.astype(np.float64) * scale[None, :]
    b1_eff = b1.astype(np.float64) + W1.astype(np.float64) @ offset
    c2 = np.tile(np.arange(H, dtype=np.float32) - 15.5, H)          # col idx
    c1 = np.repeat(np.arange(H, dtype=np.float32) - 15.5, H)         # row idx
    return {
        "w1": np.ascontiguousarray(W1_eff.T.astype(np.float32)),
        "b1": b1_eff.astype(np.float32).reshape(-1, 1),
        "w2": np.ascontiguousarray(W2.T.astype(np.float32)),
        "b2": b2.reshape(-1, 1).astype(np.float32),
        "w3": np.ascontiguousarray(W3.T.astype(np.float32)),
        "b3": b3.reshape(-1, 1).astype(np.float32),
        "w4": np.ascontiguousarray(W4.T.astype(np.float32)),
        "b4": b4.reshape(-1, 1).astype(np.float32),
        "idn": np.eye(P, dtype=np.float32),
        "cg2": np.broadcast_to(c2[None, :], (P, D)).copy(),
        "cg1": np.broadcast_to(c1[None, :], (P, D)).copy(),
        "cbias": np.broadcast_to(np.array(
            [C2X, -C2X, 0.0], np.float32)[None, :], (P, 3)).copy(),
    }


_NC_CACHE = {}


def _get_nc(n_tiles, debug_features, **kw):
    key = (n_tiles, debug_features, tuple(sorted(kw.items())))
    if key not in _NC_CACHE:
        _NC_CACHE[key] = build_nc(n_tiles, debug_features, **kw)
    return _NC_CACHE[key]


def run(t, weights, n_tiles=N_TILES, debug_features=False, trace=False, **kw):
    nc = _get_nc(n_tiles, debug_features, **kw)
    rows = TILE_IMGS * n_tiles
    in_maps = []
    for c in range(N_CORES):
        m = {"t": np.ascontiguousarray(t[c * B_CORE:c * B_CORE + rows])}
        m.update(weights)
        in_maps.append(m)
    res = run_bass_kernel_spmd(nc, in_maps, core_ids=list(range(N_CORES)),
                               trace=trace)
    outs = [r["y"] for r in res.results]
    return outs, res


def kernel(t, W1, b1, W2, b2, W3, b3, W4, b4):
    weights = _prep_weights(W1, b1, W2, b2, W3, b3, W4, b4)
    outs, _ = run(t, weights)
    y = np.concatenate(outs, axis=0)
    return np.ascontiguousarray(y.astype(np.float32))


# revision 24
# speedup vs baseline: 1.3791x; 1.0467x over previous
"""Trainium2 Bass kernel for nn_CNNModel_42064909697048.

Per-image row/col statistics (min/argmin/max/argmax/mean/median/argmedian
over both axes of each 28x28 image) -> 392 features -> 4-layer MLP ->
softmax, data-parallel over 8 NeuronCores.

Approach: values are packed into integer-exact fp32 keys
    key = 32*trunc(x*65536) + local_index
so a single min/max/rank-select on keys yields both the value and its
argindex (ties break toward the smaller index, matching numpy/torch).
Min, lower-median (rank 13) and max are produced simultaneously by one
Batcher odd-even sorting network pruned to outputs {0,13,27} (133
compare-exchanges), vectorized across 4 images x 28 groups per partition
in a position-major layout. Work is split across the Vector (axis-2 sort,
key build), GpSimd (axis-1 sort, sums) and Scalar (compare-exchange
copy-backs, activations) engines; the MLP runs on the tensor engine with
batch-512 matmuls. Index/scale corrections are folded into W1/b1.

Self-contained: hardcodes shapes/sharding; no sibling imports.
"""

import numpy as np

import concourse.bass as bass
import concourse.mybir as mybir
import concourse.tile as tile_mod
from concourse.tile import TileContext
from concourse.bass_utils import run_bass_kernel_spmd
from concourse.alu_op_type import AluOpType

# ---------------------------------------------------------------- constants
B_TOTAL = 131072
N_CORES = 8
B_CORE = B_TOTAL // N_CORES          # 16384
H = 28
D = 784
P = 128
PACK = 8                             # images per partition
TILE_IMGS = P * PACK                 # 512
N_TILES = B_CORE // TILE_IMGS        # 32
G = PACK * H                         # sort groups per partition = 112
FD = PACK * D                        # free dim of an image tile = 3136
NFEAT = 392
QS = 32768.0                         # value quantization scale (2^15)
KS = 32.0                            # index slots per quantum
C2X = float(2**23 + 2**19)           # RNE integerization bias (covers +-2^19)
F32 = mybir.dt.float32

# Batcher odd-even mergesort net for 28, pruned to outputs {0,13,27};
# stages of merged groups (d, start, (n1,s1), (n2,s2)):
# lo positions = {start + u*s1 + v*s2}, hi = lo + d.
NET28 = [[(1, 0, (14, 2), (1, 1))], [(2, 0, (7, 4), (2, 1))], [(1, 1, (7, 4), (1, 1)), (4, 0, (3, 8), (2, 3))], [(4, 1, (3, 8), (2, 1)), (8, 0, (2, 7), (1, 1)), (8, 16, (1, 1), (1, 1)), (1, 25, (1, 1), (1, 1))], [(2, 2, (3, 8), (2, 1)), (16, 0, (1, 1), (1, 1))], [(1, 1, (3, 8), (3, 2))], [(8, 1, (6, 1), (1, 1)), (8, 17, (3, 1), (1, 1)), (4, 20, (1, 1), (1, 1))], [(4, 4, (2, 17), (3, 1)), (4, 7, (1, 1), (1, 1)), (2, 18, (1, 1), (1, 1))], [(2, 2, (3, 4), (2, 1)), (2, 19, (2, 3), (1, 1)), (2, 23, (1, 1), (1, 1)), (1, 17, (1, 1), (1, 1))], [(1, 1, (2, 18), (4, 2)), (1, 9, (3, 2), (1, 1))], [(16, 1, (11, 1), (1, 1))], [(8, 8, (8, 1), (1, 1))], [(4, 7, (2, 5), (1, 1)), (4, 13, (2, 1), (1, 1)), (4, 23, (1, 1), (1, 1))], [(2, 11, (2, 3), (1, 1))], [(1, 13, (1, 1), (1, 1))]]

# feature column offsets within a 392-block (reference concat order)
OFF = {k: i * H for i, k in enumerate(
    ["min_v1", "min_i1", "min_v2", "min_i2",
     "max_v1", "max_i1", "max_v2", "max_i2",
     "mean_1", "mean_2",
     "med_v1", "med_i1", "med_v2", "med_i2"])}

# ------------------------------------------------- tile tail-drain workaround
def _patched_drain_and_barrier(self, tick_clock, wait_clock):
    drain_inst = self.nc.sync.drain()
    wait_clock.add_sem_waits(
        drain_inst.ins, tile_mod.ScopedClock({None: tick_clock.global_clock})
    )
    si = drain_inst.ins.sync_info
    waits = list(si.on_wait or [])
    if len(waits) > 1:
        si.on_wait = waits[:1]
        for w in waits[1:]:
            d2 = self.nc.sync.drain()
            si2 = d2.ins.sync_info
            if si2 is None:
                d2.ins.sync_info = mybir.SyncInfo(on_wait=[w], on_update=[])
            else:
                si2.on_wait = [w]
    self.nc.all_engine_barrier()
    assert self.sems is not None
    popped = self.nc._tile_sem_poison_stack.pop()
    assert popped is self._sem_poison
    self.nc.clear_and_free_semaphores(list(self.sems.allocated().values()))
    self.nc.all_engine_barrier()


tile_mod.TileContext._drain_and_barrier = _patched_drain_and_barrier


def _pos2d(base, n1, s1, n2, s2, d):
    """Return access plans for a merged CE group in a position-major
    [p, 28, G] view. Yields ('slc', lo_args, hi_args) per emitted op where
    args describe how to slice. Falls back to splitting when a 2D pattern
    isn't expressible as an einops view."""
    def ok1d(b, n, s):
        return (b, n, s)

    if n1 == 1 or n2 == 1:
        n, s = (n2, s2) if n1 == 1 else (n1, s1)
        yield ("1d", ok1d(base, n, s), ok1d(base + d, n, s))
        return
    # try 2D einops view: requires s2 == 1, s1 | 28, block fits
    def try2d(b):
        if s2 != 1 or 28 % s1 != 0:
            return None
        a0, b0 = b // s1, b % s1
        if b0 + n2 <= s1 and a0 + n1 <= 28 // s1:
            return (a0, b0)
        return None
    lo2, hi2 = try2d(base), try2d(base + d)
    if lo2 is not None and hi2 is not None:
        yield ("2d", (s1, lo2[0], lo2[1], n1, n2), (s1, hi2[0], hi2[1], n1, n2))
        return
    # split along the smaller axis into 1D ops
    if n1 <= n2:
        for u in range(n1):
            b = base + u * s1
            yield ("1d", ok1d(b, n2, s2), ok1d(b + d, n2, s2))
    else:
        for v in range(n2):
            b = base + v * s2
            yield ("1d", ok1d(b, n1, s1), ok1d(b + d, n1, s1))


# ------------------------------------------------------------- bass program
def build_nc(n_tiles: int = N_TILES, debug_features: bool = False):
    nc = bass.Bass()
    t_in = nc.dram_tensor("t", [TILE_IMGS * n_tiles, D], F32,
                          kind="ExternalInput")
    w1 = nc.dram_tensor("w1", [NFEAT, 270], F32, kind="ExternalInput")
    b1 = nc.dram_tensor("b1", [270, 1], F32, kind="ExternalInput")
    w2 = nc.dram_tensor("w2", [270, 90], F32, kind="ExternalInput")
    b2 = nc.dram_tensor("b2", [90, 1], F32, kind="ExternalInput")
    w3 = nc.dram_tensor("w3", [90, 30], F32, kind="ExternalInput")
    b3 = nc.dram_tensor("b3", [30, 1], F32, kind="ExternalInput")
    w4 = nc.dram_tensor("w4", [30, 10], F32, kind="ExternalInput")
    b4 = nc.dram_tensor("b4", [10, 1], F32, kind="ExternalInput")
    idn = nc.dram_tensor("idn", [P, P], F32, kind="ExternalInput")
    cg2 = nc.dram_tensor("cg2", [P, D], F32, kind="ExternalInput")  # col idx
    cg1 = nc.dram_tensor("cg1", [P, D], F32, kind="ExternalInput")  # row idx
    cbias = nc.dram_tensor("cbias", [P, 3], F32, kind="ExternalInput")
    if debug_features:
        y_out = nc.dram_tensor("y", [TILE_IMGS * n_tiles, NFEAT], F32,
                               kind="ExternalOutput")
    else:
        y_out = nc.dram_tensor("y", [TILE_IMGS * n_tiles, 10], F32,
                               kind="ExternalOutput")

    MIN = AluOpType.min
    MAX = AluOpType.max
    ADD = AluOpType.add
    SUB = AluOpType.subtract
    MUL = AluOpType.mult
    MOD = AluOpType.mod
    AXX = mybir.AxisListType.X

    with TileContext(nc) as tc:
        with (
            tc.tile_pool(name="wpool", bufs=1) as wpool,
            tc.tile_pool(name="xpool", bufs=1) as xpool,
            tc.tile_pool(name="kpool", bufs=1) as kpool,
            tc.tile_pool(name="vpool", bufs=2) as vpool,
            tc.tile_pool(name="tspool", bufs=3) as tspool,
            tc.tile_pool(name="fpool", bufs=1) as fpool,
            tc.tile_pool(name="mpool", bufs=1) as mpool,
            tc.tile_pool(name="mxpool", bufs=1) as mxpool,
            tc.tile_pool(name="psT", bufs=2, space="PSUM") as psT,
            tc.tile_pool(name="psL", bufs=2, space="PSUM") as psL,
            tc.tile_pool(name="psS", bufs=2, space="PSUM") as psS,
        ):
            # ---- static weights/consts into SBUF
            w1_t = [wpool.tile([128, 270], F32, name=f"w1_{i}", tag=f"w1_{i}")
                    for i in range(3)]
            w1_t.append(wpool.tile([8, 270], F32, name="w1_3", tag="w1_3"))
            for i in range(3):
                nc.sync.dma_start(w1_t[i][:], w1[128 * i:128 * (i + 1), :])
            nc.sync.dma_start(w1_t[3][:], w1[384:392, :])
            w2_t = [wpool.tile([128, 90], F32, name="w2_0", tag="w2_0"),
                    wpool.tile([128, 90], F32, name="w2_1", tag="w2_1"),
                    wpool.tile([14, 90], F32, name="w2_2", tag="w2_2")]
            nc.sync.dma_start(w2_t[0][:], w2[0:128, :])
            nc.sync.dma_start(w2_t[1][:], w2[128:256, :])
            nc.sync.dma_start(w2_t[2][:], w2[256:270, :])
            w3_t = wpool.tile([90, 30], F32, name="w3", tag="w3")
            nc.sync.dma_start(w3_t[:], w3[:, :])
            w4_t = wpool.tile([30, 10], F32, name="w4", tag="w4")
            nc.sync.dma_start(w4_t[:], w4[:, :])
            b1_t = [wpool.tile([128, 1], F32, name="b1_0", tag="b1_0"),
                    wpool.tile([128, 1], F32, name="b1_1", tag="b1_1"),
                    wpool.tile([14, 1], F32, name="b1_2", tag="b1_2")]
            nc.sync.dma_start(b1_t[0][:], b1[0:128, :])
            nc.sync.dma_start(b1_t[1][:], b1[128:256, :])
            nc.sync.dma_start(b1_t[2][:], b1[256:270, :])
            b2_t = wpool.tile([90, 1], F32, name="b2", tag="b2")
            nc.sync.dma_start(b2_t[:], b2[:, :])
            b3_t = wpool.tile([30, 1], F32, name="b3", tag="b3")
            nc.sync.dma_start(b3_t[:], b3[:, :])
            b4_t = wpool.tile([10, 1], F32, name="b4", tag="b4")
            nc.sync.dma_start(b4_t[:], b4[:, :])
            idn_t = wpool.tile([P, P], F32, name="idn", tag="idn")
            nc.sync.dma_start(idn_t[:], idn[:, :])
            cg2_t = wpool.tile([P, D], F32, name="cg2", tag="cg2")
            nc.sync.dma_start(cg2_t[:], cg2[:, :])
            cg1_t = wpool.tile([P, D], F32, name="cg1", tag="cg1")
            nc.sync.dma_start(cg1_t[:], cg1[:, :])
            cb_t = wpool.tile([P, 3], F32, name="cbias", tag="cbias")
            nc.sync.dma_start(cb_t[:], cbias[:, :])
            bC = cb_t[:, 0:1]        # +C2X
            bNC = cb_t[:, 1:2]       # -C2X
            cg2_b = cg2_t[:].rearrange("p (u d) -> p u d", u=1).broadcast_to(
                [P, PACK, D])
            cg1_b = cg1_t[:].rearrange("p (u d) -> p u d", u=1).broadcast_to(
                [P, PACK, D])
            IDENT = mybir.ActivationFunctionType.Identity

            m1_chunks = [(0, 128), (128, 128), (256, 14)]
            k_chunks = [(0, 128), (128, 128), (256, 128), (384, 8)]

            def prefetch(it):
                """DMA tile it and run the Act-side key integerization."""
                X = xpool.tile([P, FD], F32, name="x", tag="x")
                nc.sync.dma_start(
                    X[:], t_in[TILE_IMGS * it:TILE_IMGS * (it + 1), :]
                    .rearrange("(p i) d -> p (i d)", i=PACK))
                # round() via the RNE +-(2^23+2^19) trick on the fp32 adder.
                # Tt/Tu share the V buffers (bufs=2: tile k uses the slot
                # retired at tile k-2, so this never waits on tile k-1)
                Tt = vpool.tile([P, FD], F32, name="tt", tag="v2")
                Tu = vpool.tile([P, FD], F32, name="tu", tag="v1")
                nc.scalar.activation(Tt[:], X[:], IDENT, bias=bC, scale=QS)
                nc.scalar.activation(Tu[:], Tt[:], IDENT, bias=bNC, scale=1.0)
                return X, Tt, Tu

            nxt = prefetch(0)
            for it in range(n_tiles):
                X, Tt, Tu = nxt

                A1 = kpool.tile([P, FD], F32, name="a1", tag="a1")
                A2 = kpool.tile([P, FD], F32, name="a2", tag="a2")
                Tui = Tu[:].rearrange("p (i d) -> p i d", d=D)
                A1i4 = A1[:].rearrange("p (i d) -> p i d", d=D)
                A2i4 = A2[:].rearrange("p (i d) -> p i d", d=D)
                nc.vector.scalar_tensor_tensor(A1i4, Tui, KS, cg1_b,
                                               op0=MUL, op1=ADD)
                nc.vector.scalar_tensor_tensor(A2i4, Tui, KS, cg2_b,
                                               op0=MUL, op1=ADD)

                V2, V1 = Tt, Tu
                V2x = V2[:].rearrange("p (x g) -> p x g", g=G)
                V1x = V1[:].rearrange("p (x g) -> p x g", g=G)

                # ---- stage 0 (writes every position; no temp/copy needed)
                # axis-2: groups (i,r) stride 28, positions c
                A2x = A2[:].rearrange("p (g x) -> p x g", x=H)
                nc.vector.tensor_tensor(V2x[:, 0:27:2, :], A2x[:, 0:27:2, :],
                                        A2x[:, 1:28:2, :], op=MIN)
                nc.vector.tensor_tensor(V2x[:, 1:28:2, :], A2x[:, 0:27:2, :],
                                        A2x[:, 1:28:2, :], op=MAX)
                # axis-1: groups (i,c), positions r (merged across images)
                A1p = A1[:].rearrange("p (i r c) -> p r i c", r=H, c=H)
                V1p = V1[:].rearrange("p (x i c) -> p x i c", i=PACK, c=H)
                nc.vector.tensor_tensor(V1p[:, 0:27:2, :, :],
                                        A1p[:, 0:27:2, :, :],
                                        A1p[:, 1:28:2, :, :], op=MIN)
                nc.vector.tensor_tensor(V1p[:, 1:28:2, :, :],
                                        A1p[:, 0:27:2, :, :],
                                        A1p[:, 1:28:2, :, :], op=MAX)

                F = fpool.tile([P, PACK * NFEAT], F32, name="feat", tag="feat")
                Ff = F[:].rearrange("p (i f) -> p i f", f=NFEAT)

                # ---- remaining sort stages, interleaved axis-2/axis-1
                def mk_view(Vt, spec, arg):
                    if spec == "1d":
                        b, n, s = arg
                        vx = Vt[:].rearrange("p (x g) -> p x g", g=G)
                        return vx[:, b:b + s * (n - 1) + 1:s, :] if s > 1 \
                            else vx[:, b:b + n, :]
                    s1, a0, b0, n1, n2 = arg
                    vab = Vt[:].rearrange("p (a b g) -> p a b g", b=s1, g=G)
                    return vab[:, a0:a0 + n1, b0:b0 + n2, :]

                def emit_stage(Vt, stage, eng, ts_tag):
                    for (d, st, (n1, s1), (n2, s2)) in stage:
                        for spec, lo_a, hi_a in _pos2d(st, n1, s1, n2, s2, d):
                            lo = mk_view(Vt, spec, lo_a)
                            hi = mk_view(Vt, spec, hi_a)
                            Ts = tspool.tile([P, 11 * G], F32, name="ts",
                                             tag="ts")
                            if spec == "1d":
                                n = lo_a[1]
                                tt = Ts[:].rearrange(
                                    "p (s g) -> p s g", g=G)[:, 0:n, :]
                            else:
                                n1_, n2_ = lo_a[3], lo_a[4]
                                bb = 2 if n2_ <= 2 else 4
                                tt = Ts[:, 0:8 * G].rearrange(
                                    "p (a b g) -> p a b g", b=bb, g=G)[
                                        :, 0:n1_, 0:n2_, :]
                            eng.tensor_tensor(tt, lo, hi, op=MIN)
                            eng.tensor_tensor(hi, lo, hi, op=MAX)
                            nc.scalar.copy(lo, tt)

                for stage in NET28[1:]:
                    emit_stage(V2, stage, nc.vector, "ts2")
                    emit_stage(V1, stage, nc.vector, "ts1")

                if it + 1 < n_tiles:
                    nxt = prefetch(it + 1)

                if pending_softmax is not None:
                    pending_softmax()
                    pending_softmax = None

                # ---- sums (mean features)
                # mean_2: contiguous innermost reduce (full rate)
                Xi = X[:].rearrange("p (i r c) -> p i r c", r=H, c=H)
                o = OFF["mean_2"]
                nc.vector.tensor_reduce(Ff[:, :, o:o + H], Xi, axis=AXX,
                                        op=ADD)
                # mean_1 (column sums): binary add-tree over contiguous row
                # slices (a strided reduce runs at half rate); A1's buffer is
                # dead here and serves as scratch.
                Sv = A1[:].rearrange("p (i r c) -> p i r c", r=H, c=H)
                nc.vector.tensor_tensor(Sv[:, :, 0:14, :], Xi[:, :, 0:14, :],
                                        Xi[:, :, 14:28, :], op=ADD)
                nc.vector.tensor_tensor(Sv[:, :, 0:7, :], Sv[:, :, 0:7, :],
                                        Sv[:, :, 7:14, :], op=ADD)
                nc.vector.tensor_tensor(Sv[:, :, 0:3, :], Sv[:, :, 0:3, :],
                                        Sv[:, :, 3:6, :], op=ADD)
                nc.vector.tensor_tensor(Sv[:, :, 0:1, :], Sv[:, :, 0:1, :],
                                        Sv[:, :, 1:2, :], op=ADD)
                nc.vector.tensor_tensor(Sv[:, :, 0:1, :], Sv[:, :, 0:1, :],
                                        Sv[:, :, 2:3, :], op=ADD)
                o = OFF["mean_1"]
                F1v = Ff[:, :, o:o + H].rearrange("p i (u c) -> p i u c", u=1)
                nc.vector.tensor_tensor(F1v, Sv[:, :, 0:1, :],
                                        Sv[:, :, 6:7, :], op=ADD)

                # ---- feature extraction from key arrays
                # q = rne_int(K/32 - 15.5/32); Fv = q; Fi = K - 32q
                def extract(Vt, pos, vname, iname):
                    K = Vt[:, pos * G:(pos + 1) * G].rearrange(
                        "p (i r) -> p i r", r=H)
                    ov, oi = OFF[vname], OFF[iname]
                    Fv = Ff[:, :, ov:ov + H]
                    Fi = Ff[:, :, oi:oi + H]
                    t1 = mpool.tile([P, G], F32, name="ext1", tag="ext1")
                    t1v = t1[:].rearrange("p (i r) -> p i r", r=H)
                    nc.scalar.activation(t1v, K, IDENT, bias=bC,
                                         scale=1.0 / 32.0)
                    nc.scalar.activation(Fv, t1v, IDENT, bias=bNC, scale=1.0)
                    nc.vector.scalar_tensor_tensor(Fi, Fv, -KS, K,
                                                   op0=MUL, op1=ADD)

                extract(V2, 0, "min_v2", "min_i2")
                extract(V2, 13, "med_v2", "med_i2")
                extract(V2, 27, "max_v2", "max_i2")
                extract(V1, 0, "min_v1", "min_i1")
                extract(V1, 13, "med_v1", "med_i1")
                extract(V1, 27, "max_v1", "max_i1")

                if debug_features:
                    nc.sync.dma_start(
                        y_out[TILE_IMGS * it:TILE_IMGS * (it + 1), :]
                        .rearrange("(p i) f -> p (i f)", i=PACK), F[:])
                    continue

                # ---- MLP (batch 512 = 4 slots x 128 partitions)
                fTs = [mxpool.tile([128, TILE_IMGS], F32, name=f"fts{ci}",
                                  tag=f"fts{ci}") if kc == 128 else
                       mxpool.tile([8, TILE_IMGS], F32, name=f"fts{ci}",
                                  tag=f"fts{ci}")
                       for ci, (k0, kc) in enumerate(k_chunks)]
                for i in range(PACK):
                    for ci, (k0, kc) in enumerate(k_chunks):
                        pt = psT.tile([P, P], F32, name=f"tp{i}_{ci}",
                                      tag="tp")
                        nc.tensor.transpose(
                            pt[0:kc, :], F[:, NFEAT * i + k0:NFEAT * i + k0 + kc],
                            idn_t[:])
                        nc.scalar.copy(fTs[ci][0:kc, 128 * i:128 * (i + 1)],
                                       pt[0:kc, :])

                ex = mpool.tile([10, TILE_IMGS], F32, name="ex", tag="ex")
                for h in range(0, TILE_IMGS, 512):
                    hs = slice(h, h + 512)
                    a1 = []
                    for mi, (m0, mc) in enumerate(m1_chunks):
                        ps = psL.tile([P, 512], F32, name=f"l1_{m0}",
                                      tag="l1")[0:mc, :]
                        for ci, (k0, kc) in enumerate(k_chunks):
                            nc.tensor.matmul(ps[:], w1_t[ci][0:kc, m0:m0 + mc],
                                             fTs[ci][0:kc, hs],
                                             start=(ci == 0), stop=(ci == 3))
                        sb = mpool.tile([P, 512], F32, name=f"a1_{m0}",
                                        tag=f"a1_{m0}")[0:mc, :]
                        nc.scalar.activation(sb, ps,
                                             mybir.ActivationFunctionType.Relu,
                                             bias=b1_t[mi][0:mc, :], scale=1.0)
                        a1.append(sb)

                    ps2 = psS.tile([P, 512], F32, name="l2",
                                   tag="l2")[0:90, :]
                    for ci, (k0, kc) in enumerate(m1_chunks):
                        nc.tensor.matmul(ps2[:], w2_t[ci][0:kc, :],
                                         a1[ci][0:kc, :] if kc != 128 else a1[ci],
                                         start=(ci == 0), stop=(ci == 2))
                    a2t = mpool.tile([90, 512], F32, name="a2", tag="a2")
                    nc.scalar.activation(a2t[:], ps2,
                                         mybir.ActivationFunctionType.Relu,
                                         bias=b2_t[:], scale=1.0)

                    ps3 = psS.tile([P, 512], F32, name="l3",
                                   tag="l3")[0:30, :]
                    nc.tensor.matmul(ps3[:], w3_t[:], a2t[:], start=True,
                                     stop=True)
                    a3t = mpool.tile([30, 512], F32, name="a3", tag="a3")
                    nc.scalar.activation(a3t[:], ps3,
                                         mybir.ActivationFunctionType.Relu,
                                         bias=b3_t[:], scale=1.0)

                    ps4 = psS.tile([P, 512], F32, name="l4",
                                   tag="l2")[0:10, :]
                    nc.tensor.matmul(ps4[:], w4_t[:], a3t[:], start=True,
                                     stop=True)
                    nc.scalar.activation(ex[:, hs], ps4,
                                         mybir.ActivationFunctionType.Exp,
                                         bias=b4_t[:], scale=1.0)

                ext = mpool.tile([P, PACK * 10], F32, name="ext", tag="ext")
                for i in range(PACK):
                    pst = psT.tile([P, P], F32, name=f"sm{i}", tag="tp")
                    nc.tensor.transpose(pst[:, 0:10],
                                        ex[:, 128 * i:128 * (i + 1)],
                                        idn_t[0:10, 0:10])
                    nc.scalar.copy(ext[:, 10 * i:10 * (i + 1)], pst[:, 0:10])
                def softmax_fin(it=it, ext=ext):
                    exi = ext[:].rearrange("p (i u) -> p i u", u=10)
                    sums = mpool.tile([P, PACK], F32, name="sums", tag="sums")
                    nc.vector.tensor_reduce(sums[:], exi, axis=AXX, op=ADD)
                    rcp = mpool.tile([P, PACK], F32, name="rcp", tag="rcp")
                    nc.vector.reciprocal(rcp[:], sums[:])
                    yt = mpool.tile([P, PACK * 10], F32, name="yt", tag="yt")
                    rcb = rcp[:].rearrange("p (i u) -> p i u",
                                           u=1).broadcast_to([P, PACK, 10])
                    nc.vector.tensor_tensor(
                        yt[:].rearrange("p (i u) -> p i u", u=10),
                        exi, rcb, op=MUL)
                    nc.sync.dma_start(
                        y_out[TILE_IMGS * it:TILE_IMGS * (it + 1), :]
                        .rearrange("(p i) u -> p i u", i=PACK),
                        yt[:].rearrange("p (i u) -> p i u", u=10))
                pending_softmax = softmax_fin

            if pending_softmax is not None:
                pending_softmax()

    _split_excess_waits(nc)
    return nc


MAX_WAITS = 1


def _split_excess_waits(nc):
    """Walrus in this container rejects instructions with >MAX_WAITS sem
    waits; hoist the excess onto NoOp carriers inserted just before."""
    import bass_rust
    ctr = [0]
    for f in nc.m.functions:
        for blk in f.blocks:
            insts = list(blk.instructions)
            out = []
            changed = False
            for inst in insts:
                si = inst.sync_info
                waits = list(si.on_wait) if (si and si.on_wait) else []
                if len(waits) > MAX_WAITS:
                    changed = True
                    excess = waits[:-MAX_WAITS]
                    si.on_wait = waits[-MAX_WAITS:]
                    for k in range(0, len(excess), MAX_WAITS):
                        nop = bass_rust.InstNoOp(
                            name=f"WSPLIT-{ctr[0]}", ins=[], outs=[])
                        ctr[0] += 1
                        nop.engine = inst.engine
                        nop.sync_info = mybir.SyncInfo(
                            on_wait=excess[k:k + MAX_WAITS], on_update=[])
                        out.append(nop)
                out.append(inst)
            if changed:
                blk.instructions = out


# ------------------------------------------------------------- numpy driver
def _prep_weights(W1, b1, W2, b2, W3, b3, W4, b4):
    """Fold per-feature affine corrections into W1/b1; return transposed
    weight matrices plus constant tiles."""
    scale = np.ones(NFEAT, np.float64)
    offset = np.zeros(NFEAT, np.float64)
    for name in ("min_v1", "min_v2", "max_v1", "max_v2",
                 "med_v1", "med_v2"):
        o = OFF[name]
        scale[o:o + H] = 1.0 / QS
    for name in ("mean_1", "mean_2"):
        o = OFF[name]
        scale[o:o + H] = 1.0 / H
    for name in ("min_i1", "min_i2", "max_i1", "max_i2",
                 "med_i1", "med_i2"):
        o = OFF[name]
        offset[o:o + H] = 15.5
    W1_eff = W1.astype(np.float64) * scale[None, :]
    b1_eff = b1.astype(np.float64) + W1.astype(np.float64) @ offset
    c2 = np.tile(np.arange(H, dtype=np.float32) - 15.5, H)          # col idx
    c1 = np.repeat(np.arange(H, dtype=np.float32) - 15.5, H)         # row idx
    return {
        "w1": np.ascontiguousarray(W1_eff.T.astype(np.float32)),
        "b1": b1_eff.astype(np.float32).reshape(-1, 1),
        "w2": np.ascontiguousarray(W2.T.astype(np.float32)),
        "b2": b2.reshape(-1, 1).astype(np.float32),
        "w3": np.ascontiguousarray(W3.T.astype(np.float32)),
        "b3": b3.reshape(-1, 1).astype(np.float32),
        "w4": np.ascontiguousarray(W4.T.astype(np.float32)),
        "b4": b4.reshape(-1, 1).astype(np.float32),
        "idn": np.eye(P, dtype=np.float32),
        "cg2": np.broadcast_to(c2[None, :], (P, D)).copy(),
        "cg1": np.broadcast_to(c1[None, :], (P, D)).copy(),
        "cbias": np.broadcast_to(np.array(
            [C2X, -C2X, 0.0], np.float32)[None, :], (P, 3)).copy(),
    }


_NC_CACHE = {}


def _get_nc(n_tiles, debug_features, **kw):
    key = (n_tiles, debug_features, tuple(sorted(kw.items())))
    if key not in _NC_CACHE:
        _NC_CACHE[key] = build_nc(n_tiles, debug_features, **kw)
    return _NC_CACHE[key]


def run(t, weights, n_tiles=N_TILES, debug_features=False, trace=False, **kw):
    nc = _get_nc(n_tiles, debug_features, **kw)
    rows = TILE_IMGS * n_tiles
    in_maps = []
    for c in range(N_CORES):
        m = {"t": np.ascontiguousarray(t[c * B_CORE:c * B_CORE + rows])}
        m.update(weights)
        in_maps.append(m)
    res = run_bass_kernel_spmd(nc, in_maps, core_ids=list(range(N_CORES)),
                               trace=trace)
    outs = [r["y"] for r in res.results]
    return outs, res


def kernel(t, W1, b1, W2, b2, W3, b3, W4, b4):
    weights = _prep_weights(W1, b1, W2, b2, W3, b3, W4, b4)
    outs, _ = run(t, weights)
    y = np.concatenate(outs, axis=0)
    return np.ascontiguousarray(y.astype(np.float32))


# revision 26
# speedup vs baseline: 1.4310x; 1.0377x over previous
"""Trainium2 Bass kernel for nn_CNNModel_42064909697048.

Per-image row/col statistics (min/argmin/max/argmax/mean/median/argmedian
over both axes of each 28x28 image) -> 392 features -> 4-layer MLP ->
softmax, data-parallel over 8 NeuronCores.

Approach: values are packed into integer-exact fp32 keys
    key = 32*trunc(x*65536) + local_index
so a single min/max/rank-select on keys yields both the value and its
argindex (ties break toward the smaller index, matching numpy/torch).
Min, lower-median (rank 13) and max are produced simultaneously by one
Batcher odd-even sorting network pruned to outputs {0,13,27} (133
compare-exchanges), vectorized across 4 images x 28 groups per partition
in a position-major layout. Work is split across the Vector (axis-2 sort,
key build), GpSimd (axis-1 sort, sums) and Scalar (compare-exchange
copy-backs, activations) engines; the MLP runs on the tensor engine with
batch-512 matmuls. Index/scale corrections are folded into W1/b1.

Self-contained: hardcodes shapes/sharding; no sibling imports.
"""

import numpy as np

import concourse.bass as bass
import concourse.mybir as mybir
import concourse.tile as tile_mod
from concourse.tile import TileContext
from concourse.bass_utils import run_bass_kernel_spmd
from concourse.alu_op_type import AluOpType

# ---------------------------------------------------------------- constants
B_TOTAL = 131072
N_CORES = 8
B_CORE = B_TOTAL // N_CORES          # 16384
H = 28
D = 784
P = 128
PACK = 8                             # images per partition
TILE_IMGS = P * PACK                 # 512
N_TILES = B_CORE // TILE_IMGS        # 32
G = PACK * H                         # sort groups per partition = 112
FD = PACK * D                        # free dim of an image tile = 3136
NFEAT = 392
QS = 32768.0                         # value quantization scale (2^15)
KS = 32.0                            # index slots per quantum
C2X = float(2**23 + 2**19)           # RNE integerization bias (covers +-2^19)
F32 = mybir.dt.float32

# Batcher odd-even mergesort net for 28, pruned to outputs {0,13,27};
# stages of merged groups (d, start, (n1,s1), (n2,s2)):
# lo positions = {start + u*s1 + v*s2}, hi = lo + d.
NET28 = [[(1, 0, (14, 2), (1, 1))], [(2, 0, (7, 4), (2, 1))], [(1, 1, (7, 4), (1, 1)), (4, 0, (3, 8), (2, 3))], [(4, 1, (3, 8), (2, 1)), (8, 0, (2, 7), (1, 1)), (8, 16, (1, 1), (1, 1)), (1, 25, (1, 1), (1, 1))], [(2, 2, (3, 8), (2, 1)), (16, 0, (1, 1), (1, 1))], [(1, 1, (3, 8), (3, 2))], [(8, 1, (6, 1), (1, 1)), (8, 17, (3, 1), (1, 1)), (4, 20, (1, 1), (1, 1))], [(4, 4, (2, 17), (3, 1)), (4, 7, (1, 1), (1, 1)), (2, 18, (1, 1), (1, 1))], [(2, 2, (3, 4), (2, 1)), (2, 19, (2, 3), (1, 1)), (2, 23, (1, 1), (1, 1)), (1, 17, (1, 1), (1, 1))], [(1, 1, (2, 18), (4, 2)), (1, 9, (3, 2), (1, 1))], [(16, 1, (11, 1), (1, 1))], [(8, 8, (8, 1), (1, 1))], [(4, 7, (2, 5), (1, 1)), (4, 13, (2, 1), (1, 1)), (4, 23, (1, 1), (1, 1))], [(2, 11, (2, 3), (1, 1))], [(1, 13, (1, 1), (1, 1))]]

# feature column offsets within a 392-block (reference concat order)
OFF = {k: i * H for i, k in enumerate(
    ["min_v1", "min_i1", "min_v2", "min_i2",
     "max_v1", "max_i1", "max_v2", "max_i2",
     "mean_1", "mean_2",
     "med_v1", "med_i1", "med_v2", "med_i2"])}

# ------------------------------------------------- tile tail-drain workaround
def _patched_drain_and_barrier(self, tick_clock, wait_clock):
    drain_inst = self.nc.sync.drain()
    wait_clock.add_sem_waits(
        drain_inst.ins, tile_mod.ScopedClock({None: tick_clock.global_clock})
    )
    si = drain_inst.ins.sync_info
    waits = list(si.on_wait or [])
    if len(waits) > 1:
        si.on_wait = waits[:1]
        for w in waits[1:]:
            d2 = self.nc.sync.drain()
            si2 = d2.ins.sync_info
            if si2 is None:
                d2.ins.sync_info = mybir.SyncInfo(on_wait=[w], on_update=[])
            else:
                si2.on_wait = [w]
    self.nc.all_engine_barrier()
    assert self.sems is not None
    popped = self.nc._tile_sem_poison_stack.pop()
    assert popped is self._sem_poison
    self.nc.clear_and_free_semaphores(list(self.sems.allocated().values()))
    self.nc.all_engine_barrier()


tile_mod.TileContext._drain_and_barrier = _patched_drain_and_barrier


def _pos2d(base, n1, s1, n2, s2, d):
    """Return access plans for a merged CE group in a position-major
    [p, 28, G] view. Yields ('slc', lo_args, hi_args) per emitted op where
    args describe how to slice. Falls back to splitting when a 2D pattern
    isn't expressible as an einops view."""
    def ok1d(b, n, s):
        return (b, n, s)

    if n1 == 1 or n2 == 1:
        n, s = (n2, s2) if n1 == 1 else (n1, s1)
        yield ("1d", ok1d(base, n, s), ok1d(base + d, n, s))
        return
    # try 2D einops view: requires s2 == 1, s1 | 28, block fits
    def try2d(b):
        if s2 != 1 or 28 % s1 != 0:
            return None
        a0, b0 = b // s1, b % s1
        if b0 + n2 <= s1 and a0 + n1 <= 28 // s1:
            return (a0, b0)
        return None
    lo2, hi2 = try2d(base), try2d(base + d)
    if lo2 is not None and hi2 is not None:
        yield ("2d", (s1, lo2[0], lo2[1], n1, n2), (s1, hi2[0], hi2[1], n1, n2))
        return
    # split along the smaller axis into 1D ops
    if n1 <= n2:
        for u in range(n1):
            b = base + u * s1
            yield ("1d", ok1d(b, n2, s2), ok1d(b + d, n2, s2))
    else:
        for v in range(n2):
            b = base + v * s2
            yield ("1d", ok1d(b, n1, s1), ok1d(b + d, n1, s1))


# ------------------------------------------------------------- bass program
def build_nc(n_tiles: int = N_TILES, debug_features: bool = False):
    nc = bass.Bass()
    t_in = nc.dram_tensor("t", [TILE_IMGS * n_tiles, D], F32,
                          kind="ExternalInput")
    w1 = nc.dram_tensor("w1", [NFEAT, 270], F32, kind="ExternalInput")
    b1 = nc.dram_tensor("b1", [270, 1], F32, kind="ExternalInput")
    w2 = nc.dram_tensor("w2", [270, 90], F32, kind="ExternalInput")
    b2 = nc.dram_tensor("b2", [90, 1], F32, kind="ExternalInput")
    w3 = nc.dram_tensor("w3", [90, 30], F32, kind="ExternalInput")
    b3 = nc.dram_tensor("b3", [30, 1], F32, kind="ExternalInput")
    w4 = nc.dram_tensor("w4", [30, 10], F32, kind="ExternalInput")
    b4 = nc.dram_tensor("b4", [10, 1], F32, kind="ExternalInput")
    idn = nc.dram_tensor("idn", [P, P], F32, kind="ExternalInput")
    cg2 = nc.dram_tensor("cg2", [P, D], F32, kind="ExternalInput")  # col idx
    cg1 = nc.dram_tensor("cg1", [P, D], F32, kind="ExternalInput")  # row idx
    cbias = nc.dram_tensor("cbias", [P, 3], F32, kind="ExternalInput")
    if debug_features:
        y_out = nc.dram_tensor("y", [TILE_IMGS * n_tiles, NFEAT], F32,
                               kind="ExternalOutput")
    else:
        y_out = nc.dram_tensor("y", [TILE_IMGS * n_tiles, 10], F32,
                               kind="ExternalOutput")

    MIN = AluOpType.min
    MAX = AluOpType.max
    ADD = AluOpType.add
    SUB = AluOpType.subtract
    MUL = AluOpType.mult
    MOD = AluOpType.mod
    AXX = mybir.AxisListType.X

    with TileContext(nc) as tc:
        with (
            tc.tile_pool(name="wpool", bufs=1) as wpool,
            tc.tile_pool(name="xpool", bufs=1) as xpool,
            tc.tile_pool(name="kpool", bufs=1) as kpool,
            tc.tile_pool(name="vpool", bufs=2) as vpool,
            tc.tile_pool(name="tspool", bufs=3) as tspool,
            tc.tile_pool(name="fpool", bufs=1) as fpool,
            tc.tile_pool(name="mpool", bufs=1) as mpool,
            tc.tile_pool(name="mxpool", bufs=1) as mxpool,
            tc.tile_pool(name="psT", bufs=2, space="PSUM") as psT,
            tc.tile_pool(name="psL", bufs=2, space="PSUM") as psL,
            tc.tile_pool(name="psS", bufs=2, space="PSUM") as psS,
        ):
            # ---- static weights/consts into SBUF
            w1_t = [wpool.tile([128, 270], F32, name=f"w1_{i}", tag=f"w1_{i}")
                    for i in range(3)]
            w1_t.append(wpool.tile([8, 270], F32, name="w1_3", tag="w1_3"))
            for i in range(3):
                nc.sync.dma_start(w1_t[i][:], w1[128 * i:128 * (i + 1), :])
            nc.sync.dma_start(w1_t[3][:], w1[384:392, :])
            w2_t = [wpool.tile([128, 90], F32, name="w2_0", tag="w2_0"),
                    wpool.tile([128, 90], F32, name="w2_1", tag="w2_1"),
                    wpool.tile([14, 90], F32, name="w2_2", tag="w2_2")]
            nc.sync.dma_start(w2_t[0][:], w2[0:128, :])
            nc.sync.dma_start(w2_t[1][:], w2[128:256, :])
            nc.sync.dma_start(w2_t[2][:], w2[256:270, :])
            w3_t = wpool.tile([90, 30], F32, name="w3", tag="w3")
            nc.sync.dma_start(w3_t[:], w3[:, :])
            w4_t = wpool.tile([30, 10], F32, name="w4", tag="w4")
            nc.sync.dma_start(w4_t[:], w4[:, :])
            b1_t = [wpool.tile([128, 1], F32, name="b1_0", tag="b1_0"),
                    wpool.tile([128, 1], F32, name="b1_1", tag="b1_1"),
                    wpool.tile([14, 1], F32, name="b1_2", tag="b1_2")]
            nc.sync.dma_start(b1_t[0][:], b1[0:128, :])
            nc.sync.dma_start(b1_t[1][:], b1[128:256, :])
            nc.sync.dma_start(b1_t[2][:], b1[256:270, :])
            b2_t = wpool.tile([90, 1], F32, name="b2", tag="b2")
            nc.sync.dma_start(b2_t[:], b2[:, :])
            b3_t = wpool.tile([30, 1], F32, name="b3", tag="b3")
            nc.sync.dma_start(b3_t[:], b3[:, :])
            b4_t = wpool.tile([10, 1], F32, name="b4", tag="b4")
            nc.sync.dma_start(b4_t[:], b4[:, :])
            idn_t = wpool.tile([P, P], F32, name="idn", tag="idn")
            nc.sync.dma_start(idn_t[:], idn[:, :])
            cg2_t = wpool.tile([P, D], F32, name="cg2", tag="cg2")
            nc.sync.dma_start(cg2_t[:], cg2[:, :])
            cg1_t = wpool.tile([P, D], F32, name="cg1", tag="cg1")
            nc.sync.dma_start(cg1_t[:], cg1[:, :])
            cb_t = wpool.tile([P, 3], F32, name="cbias", tag="cbias")
            nc.sync.dma_start(cb_t[:], cbias[:, :])
            bC = cb_t[:, 0:1]        # +C2X
            bNC = cb_t[:, 1:2]       # -C2X
            cg2_b = cg2_t[:].rearrange("p (u d) -> p u d", u=1).broadcast_to(
                [P, PACK, D])
            cg1_b = cg1_t[:].rearrange("p (u d) -> p u d", u=1).broadcast_to(
                [P, PACK, D])
            IDENT = mybir.ActivationFunctionType.Identity

            m1_chunks = [(0, 128), (128, 128), (256, 14)]
            k_chunks = [(0, 128), (128, 128), (256, 128), (384, 8)]

            def prefetch(it):
                """DMA tile it and run the Act-side key integerization."""
                X = xpool.tile([P, FD], F32, name="x", tag="x")
                nc.sync.dma_start(
                    X[:], t_in[TILE_IMGS * it:TILE_IMGS * (it + 1), :]
                    .rearrange("(p i) d -> p (i d)", i=PACK))
                # round() via the RNE +-(2^23+2^19) trick on the fp32 adder.
                # Tt/Tu share the V buffers (bufs=2: tile k uses the slot
                # retired at tile k-2, so this never waits on tile k-1)
                Tt = vpool.tile([P, FD], F32, name="tt", tag="v2")
                Tu = vpool.tile([P, FD], F32, name="tu", tag="v1")
                nc.scalar.activation(Tt[:], X[:], IDENT, bias=bC, scale=QS)
                nc.scalar.activation(Tu[:], Tt[:], IDENT, bias=bNC, scale=1.0)
                return X, Tt, Tu

            nxt = prefetch(0)
            for it in range(n_tiles):
                X, Tt, Tu = nxt

                A1 = kpool.tile([P, FD], F32, name="a1", tag="a1")
                A2 = kpool.tile([P, FD], F32, name="a2", tag="a2")
                Tui = Tu[:].rearrange("p (i d) -> p i d", d=D)
                A1i4 = A1[:].rearrange("p (i d) -> p i d", d=D)
                A2i4 = A2[:].rearrange("p (i d) -> p i d", d=D)
                nc.vector.scalar_tensor_tensor(A1i4, Tui, KS, cg1_b,
                                               op0=MUL, op1=ADD)
                nc.vector.scalar_tensor_tensor(A2i4, Tui, KS, cg2_b,
                                               op0=MUL, op1=ADD)

                V2, V1 = Tt, Tu
                V2x = V2[:].rearrange("p (x g) -> p x g", g=G)
                V1x = V1[:].rearrange("p (x g) -> p x g", g=G)

                # ---- stage 0 (writes every position; no temp/copy needed)
                # axis-2: groups (i,r) stride 28, positions c
                A2x = A2[:].rearrange("p (g x) -> p x g", x=H)
                nc.vector.tensor_tensor(V2x[:, 0:27:2, :], A2x[:, 0:27:2, :],
                                        A2x[:, 1:28:2, :], op=MIN)
                nc.vector.tensor_tensor(V2x[:, 1:28:2, :], A2x[:, 0:27:2, :],
                                        A2x[:, 1:28:2, :], op=MAX)
                # axis-1: groups (i,c), positions r (merged across images)
                A1p = A1[:].rearrange("p (i r c) -> p r i c", r=H, c=H)
                V1p = V1[:].rearrange("p (x i c) -> p x i c", i=PACK, c=H)
                nc.vector.tensor_tensor(V1p[:, 0:27:2, :, :],
                                        A1p[:, 0:27:2, :, :],
                                        A1p[:, 1:28:2, :, :], op=MIN)
                nc.vector.tensor_tensor(V1p[:, 1:28:2, :, :],
                                        A1p[:, 0:27:2, :, :],
                                        A1p[:, 1:28:2, :, :], op=MAX)

                F = fpool.tile([P, PACK * NFEAT], F32, name="feat", tag="feat")
                Ff = F[:].rearrange("p (i f) -> p i f", f=NFEAT)

                # ---- sums (mean features) from KEYS: sum(A2) = 32*sum(q)
                # + linear-in-column constant, folded into W1. Reading A2
                # instead of X frees X right after Tt, so the next tile's
                # DMA overlaps the whole tile (kills the boundary stall).
                Ai = A2[:].rearrange("p (i r c) -> p i r c", r=H, c=H)
                o = OFF["mean_2"]
                nc.vector.tensor_reduce(Ff[:, :, o:o + H], Ai, axis=AXX,
                                        op=ADD)
                # mean_1 (column sums): contiguous binary add-tree; A1's
                # buffer is dead after stage 0 and serves as scratch.
                Sv = A1[:].rearrange("p (i r c) -> p i r c", r=H, c=H)
                nc.vector.tensor_tensor(Sv[:, :, 0:14, :], Ai[:, :, 0:14, :],
                                        Ai[:, :, 14:28, :], op=ADD)
                nc.vector.tensor_tensor(Sv[:, :, 0:7, :], Sv[:, :, 0:7, :],
                                        Sv[:, :, 7:14, :], op=ADD)
                nc.vector.tensor_tensor(Sv[:, :, 0:3, :], Sv[:, :, 0:3, :],
                                        Sv[:, :, 3:6, :], op=ADD)
                nc.vector.tensor_tensor(Sv[:, :, 0:1, :], Sv[:, :, 0:1, :],
                                        Sv[:, :, 1:2, :], op=ADD)
                nc.vector.tensor_tensor(Sv[:, :, 0:1, :], Sv[:, :, 0:1, :],
                                        Sv[:, :, 2:3, :], op=ADD)
                o = OFF["mean_1"]
                F1v = Ff[:, :, o:o + H].rearrange("p i (u c) -> p i u c", u=1)
                nc.vector.tensor_tensor(F1v, Sv[:, :, 0:1, :],
                                        Sv[:, :, 6:7, :], op=ADD)


                # ---- remaining sort stages, interleaved axis-2/axis-1
                def mk_view(Vt, spec, arg):
                    if spec == "1d":
                        b, n, s = arg
                        vx = Vt[:].rearrange("p (x g) -> p x g", g=G)
                        return vx[:, b:b + s * (n - 1) + 1:s, :] if s > 1 \
                            else vx[:, b:b + n, :]
                    s1, a0, b0, n1, n2 = arg
                    vab = Vt[:].rearrange("p (a b g) -> p a b g", b=s1, g=G)
                    return vab[:, a0:a0 + n1, b0:b0 + n2, :]

                def emit_stage(Vt, stage, eng, ts_tag):
                    for (d, st, (n1, s1), (n2, s2)) in stage:
                        for spec, lo_a, hi_a in _pos2d(st, n1, s1, n2, s2, d):
                            lo = mk_view(Vt, spec, lo_a)
                            hi = mk_view(Vt, spec, hi_a)
                            Ts = tspool.tile([P, 11 * G], F32, name="ts",
                                             tag="ts")
                            if spec == "1d":
                                n = lo_a[1]
                                tt = Ts[:].rearrange(
                                    "p (s g) -> p s g", g=G)[:, 0:n, :]
                            else:
                                n1_, n2_ = lo_a[3], lo_a[4]
                                bb = 2 if n2_ <= 2 else 4
                                tt = Ts[:, 0:8 * G].rearrange(
                                    "p (a b g) -> p a b g", b=bb, g=G)[
                                        :, 0:n1_, 0:n2_, :]
                            eng.tensor_tensor(tt, lo, hi, op=MIN)
                            eng.tensor_tensor(hi, lo, hi, op=MAX)
                            nc.scalar.copy(lo, tt)

                for stage in NET28[1:]:
                    emit_stage(V2, stage, nc.vector, "ts2")
                    emit_stage(V1, stage, nc.vector, "ts1")

                if it + 1 < n_tiles:
                    nxt = prefetch(it + 1)

                if pending_softmax is not None:
                    pending_softmax()
                    pending_softmax = None

                # ---- feature extraction from key arrays
                # q = rne_int(K/32 - 15.5/32); Fv = q; Fi = K - 32q
                def extract(Vt, pos, vname, iname):
                    K = Vt[:, pos * G:(pos + 1) * G].rearrange(
                        "p (i r) -> p i r", r=H)
                    ov, oi = OFF[vname], OFF[iname]
                    Fv = Ff[:, :, ov:ov + H]
                    Fi = Ff[:, :, oi:oi + H]
                    t1 = mpool.tile([P, G], F32, name="ext1", tag="ext1")
                    t1v = t1[:].rearrange("p (i r) -> p i r", r=H)
                    nc.scalar.activation(t1v, K, IDENT, bias=bC,
                                         scale=1.0 / 32.0)
                    nc.scalar.activation(Fv, t1v, IDENT, bias=bNC, scale=1.0)
                    nc.vector.scalar_tensor_tensor(Fi, Fv, -KS, K,
                                                   op0=MUL, op1=ADD)

                extract(V2, 0, "min_v2", "min_i2")
                extract(V2, 13, "med_v2", "med_i2")
                extract(V2, 27, "max_v2", "max_i2")
                extract(V1, 0, "min_v1", "min_i1")
                extract(V1, 13, "med_v1", "med_i1")
                extract(V1, 27, "max_v1", "max_i1")

                if debug_features:
                    nc.sync.dma_start(
                        y_out[TILE_IMGS * it:TILE_IMGS * (it + 1), :]
                        .rearrange("(p i) f -> p (i f)", i=PACK), F[:])
                    continue

                # ---- MLP (batch 512 = 4 slots x 128 partitions)
                fTs = [mxpool.tile([128, TILE_IMGS], F32, name=f"fts{ci}",
                                  tag=f"fts{ci}") if kc == 128 else
                       mxpool.tile([8, TILE_IMGS], F32, name=f"fts{ci}",
                                  tag=f"fts{ci}")
                       for ci, (k0, kc) in enumerate(k_chunks)]
                for i in range(PACK):
                    for ci, (k0, kc) in enumerate(k_chunks):
                        pt = psT.tile([P, P], F32, name=f"tp{i}_{ci}",
                                      tag="tp")
                        nc.tensor.transpose(
                            pt[0:kc, :], F[:, NFEAT * i + k0:NFEAT * i + k0 + kc],
                            idn_t[:])
                        nc.scalar.copy(fTs[ci][0:kc, 128 * i:128 * (i + 1)],
                                       pt[0:kc, :])

                ex = mpool.tile([10, TILE_IMGS], F32, name="ex", tag="ex")
                for h in range(0, TILE_IMGS, 512):
                    hs = slice(h, h + 512)
                    a1 = []
                    for mi, (m0, mc) in enumerate(m1_chunks):
                        ps = psL.tile([P, 512], F32, name=f"l1_{m0}",
                                      tag="l1")[0:mc, :]
                        for ci, (k0, kc) in enumerate(k_chunks):
                            nc.tensor.matmul(ps[:], w1_t[ci][0:kc, m0:m0 + mc],
                                             fTs[ci][0:kc, hs],
                                             start=(ci == 0), stop=(ci == 3))
                        sb = mpool.tile([P, 512], F32, name=f"a1_{m0}",
                                        tag=f"a1_{m0}")[0:mc, :]
                        nc.scalar.activation(sb, ps,
                                             mybir.ActivationFunctionType.Relu,
                                             bias=b1_t[mi][0:mc, :], scale=1.0)
                        a1.append(sb)

                    ps2 = psS.tile([P, 512], F32, name="l2",
                                   tag="l2")[0:90, :]
                    for ci, (k0, kc) in enumerate(m1_chunks):
                        nc.tensor.matmul(ps2[:], w2_t[ci][0:kc, :],
                                         a1[ci][0:kc, :] if kc != 128 else a1[ci],
                                         start=(ci == 0), stop=(ci == 2))
                    a2t = mpool.tile([90, 512], F32, name="a2", tag="a2")
                    nc.scalar.activation(a2t[:], ps2,
                                         mybir.ActivationFunctionType.Relu,
                                         bias=b2_t[:], scale=1.0)

                    ps3 = psS.tile([P, 512], F32, name="l3",
                                   tag="l3")[0:30, :]
                    nc.tensor.matmul(ps3[:], w3_t[:], a2t[:], start=True,
                                     stop=True)
                    a3t = mpool.tile([30, 512], F32, name="a3", tag="a3")
                    nc.scalar.activation(a3t[:], ps3,
                                         mybir.ActivationFunctionType.Relu,
                                         bias=b3_t[:], scale=1.0)

                    ps4 = psS.tile([P, 512], F32, name="l4",
                                   tag="l2")[0:10, :]
                    nc.tensor.matmul(ps4[:], w4_t[:], a3t[:], start=True,
                                     stop=True)
                    nc.scalar.activation(ex[:, hs], ps4,
                                         mybir.ActivationFunctionType.Exp,
                                         bias=b4_t[:], scale=1.0)

                ext = mpool.tile([P, PACK * 10], F32, name="ext", tag="ext")
                for i in range(PACK):
                    pst = psT.tile([P, P], F32, name=f"sm{i}", tag="tp")
                    nc.tensor.transpose(pst[:, 0:10],
                                        ex[:, 128 * i:128 * (i + 1)],
                                        idn_t[0:10, 0:10])
                    nc.scalar.copy(ext[:, 10 * i:10 * (i + 1)], pst[:, 0:10])
                def softmax_fin(it=it, ext=ext):
                    exi = ext[:].rearrange("p (i u) -> p i u", u=10)
                    sums = mpool.tile([P, PACK], F32, name="sums", tag="sums")
                    nc.vector.tensor_reduce(sums[:], exi, axis=AXX, op=ADD)
                    rcp = mpool.tile([P, PACK], F32, name="rcp", tag="rcp")
                    nc.vector.reciprocal(rcp[:], sums[:])
                    yt = mpool.tile([P, PACK * 10], F32, name="yt", tag="yt")
                    rcb = rcp[:].rearrange("p (i u) -> p i u",
                                           u=1).broadcast_to([P, PACK, 10])
                    nc.vector.tensor_tensor(
                        yt[:].rearrange("p (i u) -> p i u", u=10),
                        exi, rcb, op=MUL)
                    nc.sync.dma_start(
                        y_out[TILE_IMGS * it:TILE_IMGS * (it + 1), :]
                        .rearrange("(p i) u -> p i u", i=PACK),
                        yt[:].rearrange("p (i u) -> p i u", u=10))
                pending_softmax = softmax_fin

            if pending_softmax is not None:
                pending_softmax()

    _split_excess_waits(nc)
    return nc


MAX_WAITS = 1


def _split_excess_waits(nc):
    """Walrus in this container rejects instructions with >MAX_WAITS sem
    waits; hoist the excess onto NoOp carriers inserted just before."""
    import bass_rust
    ctr = [0]
    for f in nc.m.functions:
        for blk in f.blocks:
            insts = list(blk.instructions)
            out = []
            changed = False
            for inst in insts:
                si = inst.sync_info
                waits = list(si.on_wait) if (si and si.on_wait) else []
                if len(waits) > MAX_WAITS:
                    changed = True
                    excess = waits[:-MAX_WAITS]
                    si.on_wait = waits[-MAX_WAITS:]
                    for k in range(0, len(excess), MAX_WAITS):
                        nop = bass_rust.InstNoOp(
                            name=f"WSPLIT-{ctr[0]}", ins=[], outs=[])
                        ctr[0] += 1
                        nop.engine = inst.engine
                        nop.sync_info = mybir.SyncInfo(
                            on_wait=excess[k:k + MAX_WAITS], on_update=[])
                        out.append(nop)
                out.append(inst)
            if changed:
                blk.instructions = out


# ------------------------------------------------------------- numpy driver
def _prep_weights(W1, b1, W2, b2, W3, b3, W4, b4):
    """Fold per-feature affine corrections into W1/b1; return transposed
    weight matrices plus constant tiles."""
    scale = np.ones(NFEAT, np.float64)
    offset = np.zeros(NFEAT, np.float64)
    for name in ("min_v1", "min_v2", "max_v1", "max_v2",
                 "med_v1", "med_v2"):
        o = OFF[name]
        scale[o:o + H] = 1.0 / QS
    idx28 = np.arange(H, dtype=np.float64)
    o = OFF["mean_1"]
    scale[o:o + H] = 1.0 / (KS * QS * H)
    offset[o:o + H] = -(idx28 - 15.5) / (KS * QS)
    o = OFF["mean_2"]
    scale[o:o + H] = 1.0 / (KS * QS * H)
    offset[o:o + H] = 56.0 / (KS * QS * H)
    for name in ("min_i1", "min_i2", "max_i1", "max_i2",
                 "med_i1", "med_i2"):
        o = OFF[name]
        offset[o:o + H] = 15.5
    W1_eff = W1.astype(np.float64) * scale[None, :]
    b1_eff = b1.astype(np.float64) + W1.astype(np.float64) @ offset
    c2 = np.tile(np.arange(H, dtype=np.float32) - 15.5, H)          # col idx
    c1 = np.repeat(np.arange(H, dtype=np.float32) - 15.5, H)         # row idx
    return {
        "w1": np.ascontiguousarray(W1_eff.T.astype(np.float32)),
        "b1": b1_eff.astype(np.float32).reshape(-1, 1),
        "w2": np.ascontiguousarray(W2.T.astype(np.float32)),
        "b2": b2.reshape(-1, 1).astype(np.float32),
        "w3": np.ascontiguousarray(W3.T.astype(np.float32)),
        "b3": b3.reshape(-1, 1).astype(np.float32),
        "w4": np.ascontiguousarray(W4.T.astype(np.float32)),
        "b4": b4.reshape(-1, 1).astype(np.float32),
        "idn": np.eye(P, dtype=np.float32),
        "cg2": np.broadcast_to(c2[None, :], (P, D)).copy(),
        "cg1": np.broadcast_to(c1[None, :], (P, D)).copy(),
        "cbias": np.broadcast_to(np.array(
            [C2X, -C2X, 0.0], np.float32)[None, :], (P, 3)).copy(),
    }


_NC_CACHE = {}


def _get_nc(n_tiles, debug_features, **kw):
    key = (n_tiles, debug_features, tuple(sorted(kw.items())))
    if key not in _NC_CACHE:
        _NC_CACHE[key] = build_nc(n_tiles, debug_features, **kw)
    return _NC_CACHE[key]


def run(t, weights, n_tiles=N_TILES, debug_features=False, trace=False, **kw):
    nc = _get_nc(n_tiles, debug_features, **kw)
    rows = TILE_IMGS * n_tiles
    in_maps = []
    for c in range(N_CORES):
        m = {"t": np.ascontiguousarray(t[c * B_CORE:c * B_CORE + rows])}
        m.update(weights)
        in_maps.append(m)
    res = run_bass_kernel_spmd(nc, in_maps, core_ids=list(range(N_CORES)),
                               trace=trace)
    outs = [r["y"] for r in res.results]
    return outs, res


def kernel(t, W1, b1, W2, b2, W3, b3, W4, b4):
    weights = _prep_weights(W1, b1, W2, b2, W3, b3, W4, b4)
    outs, _ = run(t, weights)
    y = np.concatenate(outs, axis=0)
    return np.ascontiguousarray(y.astype(np.float32))
